# revision 16
# baseline (speedup 1.0000x reference)
"""AnswerDecoder (LSTM decoder w/ visual attention) on 8 TRN2 NeuronCores.

Strategy: pure data-parallel over batch (8 batches/core), zero collectives.

v2 design notes (all relative to the measured v1 trace: 787us, PE-bound):
- "Quad" layout: partition 32j+b holds (hidden-slice j, batch b). All LSTM
  cell elementwise work runs as single [104, N] instructions instead of 4-8
  [8, N] ones (ACT/DVE cost is free-dim-dominated).
- 4-way PE column tiling: the four quad groups' matmuls use tile_position
  (0, 32j) and stream concurrently (measured 82.5 ns/MM for N=512 bf16 vs
  231.8 serial). Same-bank disjoint-partition accumulation verified OK.
- bf16 weights/activations everywhere on the PE (f32r at N<256 runs at 1/4
  rate; bf16 is 1 cycle/row always). c-state and PSUM stay f32.
- One batched PE transpose per h/att/o per step ([104,128] -> [128,128] via
  zero-padded identity) instead of 4 narrow transposes each.
- Softmax without max-subtraction (|e| << 80 so fp32 exp is safe); the
  block-diagonal mask is folded into the e-matmul as a K=8 identity wave;
  row sums come free from the Exp accumulator; normalization happens on the
  exp output before transposing.
- W_u bias enters via a ones-row in abd x b_u row in pbd; vocab bias is
  added on the host.
- Vocab projection: M-tile 0 (steps 0-15) is interleaved into PE idle slots
  of steps 16-31; only M-tile 1 runs after the loop. Output is bf16
  (upcast on host).
"""

import numpy as np

B, T, R = 64, 32, 49
LOCAL, QVEC, EMB, HID, VOCAB = 1024, 512, 256, 512, 10000
START_IDX = 1
NCORES = 8
BL = B // NCORES        # 8 batches per core
ROWS = T * BL           # 256 output rows per core, t-major (row = t*BL + b)
NEG = -60000.0          # mask value; exp(NEG + e) underflows to 0 in fp32


def _quad_perm_scale():
    """Device gate-column order: group j, gate [g,i,f,og], offset f.
    Returns (perm, scale): device col -> ref 4H row, and the 0.5 tanh-half
    scaling for i/f/og. Order [g,i,...] lets the cell tanh split into two
    contiguous 256-col ops with (g,i) first so t1 starts early."""
    # ref row ranges: i 0:512, f 512:1024, g 1024:1536, o 1536:2048
    base = {0: 1024, 1: 0, 2: 512, 3: 1536}      # device gate idx -> ref base
    perm = np.empty(2048, np.int64)
    scale = np.empty(2048, np.float32)
    for j in range(4):
        for g in range(4):
            cols = slice(j * 512 + g * 128, j * 512 + g * 128 + 128)
            perm[cols] = base[g] + 128 * j + np.arange(128)
            scale[cols] = 1.0 if g == 0 else 0.5
    return perm, scale


def prep_inputs(image_local_features, image_global_features, question_vectors,
                answers, emb, W_g2o, b_g2o, W_h, W_c, W_ih, W_hh, b_ih, b_hh,
                W_attn, W_u, b_u, W_vocab, b_vocab):
    """Host-side data layout prep. Returns list of per-core input dicts."""
    import ml_dtypes
    bf16 = ml_dtypes.bfloat16
    f32 = np.float32

    L = np.asarray(image_local_features, f32)                   # [B,R,F]
    g = np.asarray(image_global_features, f32)                  # [B,2F]
    q = np.asarray(question_vectors, f32)                       # [B,Q]
    ans = np.asarray(answers).astype(np.int64)                  # [B,T]
    emb = np.asarray(emb, f32)

    perm, qscale = _quad_perm_scale()
    # recurrent weights: K = [o (512); h (512)], h-part halved (h' = 2h)
    W_cat = np.concatenate([W_ih[:, EMB:EMB + HID], 0.5 * np.asarray(W_hh, f32)],
                           axis=1)                              # [2048, 1024]
    wrq = (W_cat.T[:, perm] * qscale[None, :]).astype(bf16)     # [1024, 2048]
    wy_full = np.concatenate([np.asarray(W_ih, f32)[:, :EMB].T,
                              (np.asarray(b_ih, f32) + np.asarray(b_hh, f32))[None, :]],
                             axis=0)                            # [257, 2048ref]
    wyq = (wy_full[:, perm] * qscale[None, :]).astype(bf16)     # [257, 2048]

    whq = (2.0 * np.asarray(W_h, f32).T).astype(bf16)           # [512, 512]
    wcq = (2.0 * np.asarray(W_c, f32).T).astype(bf16)           # [512, 512]
    wgq = np.concatenate([np.asarray(W_g2o, f32).T,
                          np.asarray(b_g2o, f32)[None, :]], 0).astype(bf16)  # [2049,512]
    watq = (0.5 * np.asarray(W_attn, f32).T).astype(bf16)       # [1024, 512]
    wuaq = np.ascontiguousarray(np.asarray(W_u, f32)[:, :LOCAL].T).astype(bf16)
    wuhq = (0.5 * np.asarray(W_u, f32)[:, LOCAL:].T).astype(bf16)  # [512, 512]
    buq = np.asarray(b_u, f32)[None, :].astype(bf16)            # [1, 512]
    wv = np.ascontiguousarray(np.asarray(W_vocab, f32).T).astype(bf16)  # [512,10000]

    # col 2R is an epsilon column (-55 -> exp ~ 1.3e-24): keeps every row's
    # exp-sum nonzero so 1/ssum stays finite on fully-masked (off-diagonal)
    # rows; abd only consumes cols 0:2R so it never reaches vo.
    maskq = np.full((BL, 4, 2 * R + 1), NEG, f32)
    maskq[:, :, 2 * R] = -55.0
    for j in range(4):
        maskq[2 * j, j, 0:R] = 0.0
        maskq[2 * j + 1, j, R:2 * R] = 0.0
    maskq = maskq.astype(bf16)
    idb = np.eye(128, dtype=f32).astype(bf16)
    ones8 = np.ones((1, BL), f32).astype(bf16)

    # teacher-forced input embeddings: y_seq[t] = emb[ans[:, t-1]], y_seq[0]=emb[1]
    idx = np.concatenate([np.full((B, 1), START_IDX, np.int64), ans[:, :-1]], 1)
    y_emb = emb[idx]                                            # [B,T,EMB]

    shared = {
        "wyq": wyq, "wrq": wrq, "whq": whq, "wcq": wcq, "wgq": wgq,
        "watq": watq, "wuaq": wuaq, "wuhq": wuhq, "buq": buq, "wv": wv,
        "maskq": maskq, "idb": idb, "ones8": ones8,
    }
    in_maps = []
    for c in range(NCORES):
        s = slice(c * BL, (c + 1) * BL)
        ye = y_emb[s].transpose(1, 0, 2).reshape(ROWS, EMB)     # [(t,b), EMB]
        yT = np.concatenate([ye.T, np.ones((1, ROWS), f32)], 0).astype(bf16)
        lT = np.ascontiguousarray(L[s].reshape(BL * R, LOCAL).T).astype(bf16)
        qT = np.ascontiguousarray(q[s].T).astype(bf16)          # [512,8]
        gT = np.concatenate([g[s].T, np.ones((1, BL), f32)], 0).astype(bf16)
        d = {"yT": yT, "lT": lT, "qT": qT, "gT": gT}
        d.update(shared)
        in_maps.append(d)
    return in_maps


def build_nc():
    """Build the per-core Bass graph (identical on all 8 cores)."""
    from concourse import bacc, tile
    import concourse.mybir as mybir

    f32 = mybir.dt.float32
    bf16 = mybir.dt.bfloat16
    AF = mybir.ActivationFunctionType
    ALU = mybir.AluOpType

    nc = bacc.Bacc("TRN2", target_bir_lowering=False, debug=False,
                   num_devices=NCORES)

    def dparam(name, shape, dt=bf16):
        return nc.dram_tensor(name, list(shape), dt, kind="ExternalInput").ap()

    yT_d = dparam("yT", [257, 256])
    wyq_d = dparam("wyq", [257, 2048])
    wrq_d = dparam("wrq", [1024, 2048])
    lT_d = dparam("lT", [1024, BL * R])
    watq_d = dparam("watq", [1024, 512])
    wuaq_d = dparam("wuaq", [1024, 512])
    wuhq_d = dparam("wuhq", [512, 512])
    qT_d = dparam("qT", [512, BL])
    whq_d = dparam("whq", [512, 512])
    wcq_d = dparam("wcq", [512, 512])
    gT_d = dparam("gT", [2049, BL])
    wgq_d = dparam("wgq", [2049, 512])
    buq_d = dparam("buq", [1, 512])
    wv_d = dparam("wv", [512, VOCAB])
    mask_d = dparam("maskq", [BL, 4, 2 * R + 1])
    idb_d = dparam("idb", [128, 128])
    ones8_d = dparam("ones8", [1, BL])
    out_d = nc.dram_tensor("out", [ROWS, VOCAB], bf16, kind="ExternalOutput").ap()

    mm = nc.tensor.matmul
    ADD, MULT = ALU.add, ALU.mult
    stt = nc.vector.scalar_tensor_tensor
    vcp = nc.vector.tensor_copy
    P104 = 104  # 3*32 + 8: spans all four quad groups

    with tile.TileContext(nc) as tc:
        import contextlib
        stack = contextlib.ExitStack()
        with stack:
            pers = stack.enter_context(tc.tile_pool(name="pers", bufs=1))
            stb = stack.enter_context(tc.tile_pool(name="stb", bufs=2))
            gyp = stack.enter_context(tc.tile_pool(name="gyp", bufs=2))
            gyd = stack.enter_context(tc.tile_pool(name="gyd", bufs=1, space="DRAM"))
            pg = stack.enter_context(tc.tile_pool(name="pg", bufs=2, space="PSUM"))
            ptp = stack.enter_context(tc.tile_pool(name="ptp", bufs=2, space="PSUM"))

            dma = nc.sync.dma_start

            # ---- persistent SBUF ----
            wr_sb = pers.tile([128, 8, 2048], bf16)
            wuh_sb = pers.tile([128, 4, 512], bf16)
            apT_sb = pers.tile([128, 4, BL * R], bf16)
            pbd_sb = pers.tile([128, 4, 512], bf16)
            otb_sb = pers.tile([128, 4, T + 1, BL], bf16)
            wv_sb = pers.tile([128, 4, VOCAB], bf16)
            cq_sb = pers.tile([128, 128], f32)
            abd_sb = pers.tile([128, 4, 32], bf16)
            mask_sb = pers.tile([BL, 4, 2 * R + 1], bf16)
            idb_sb = pers.tile([128, 128], bf16)
            gy_dram = gyd.tile([ROWS, 2048], bf16)

            dma(out=mask_sb[:, :, :], in_=mask_d[:, :, :])
            dma(out=idb_sb[:, :], in_=idb_d[:, :])
            dma(out=abd_sb[2 * R:2 * R + 1, 0, 0:BL], in_=ones8_d[:, :])
            dma(out=pbd_sb[2 * R:2 * R + 1, 0, :], in_=buq_d[:, :])
            for k in range(4):
                dma(out=wuh_sb[:, k, :], in_=wuhq_d[128 * k:128 * (k + 1), :])
            for k in range(8):
                dma(out=wr_sb[:, k, :], in_=wrq_d[128 * k:128 * (k + 1), :])

            id8 = idb_sb[0:8, 0:8]
            idT = idb_sb[0:P104, 0:128]   # zero-padded transpose identity

            def quad_mm(out_t, lhsT, rhs, j, start, stop):
                mm(out_t[32 * j:32 * j + BL, :], lhsT, rhs, start=start,
                   stop=stop, skip_group_check=True, tile_position=(0, 32 * j))

            def transpose104(in_ap, src_pool=None):
                """[104, F] -> ptp tile [128, 4, 32] (cols 32k+b valid)."""
                tp = ptp.tile([128, 4, 32], bf16, name="tp", tag="tp")
                nP = in_ap.shape[0]
                nc.tensor.transpose(tp[0:in_ap.shape[1], :, :], in_ap,
                                    idb_sb[0:nP, 0:128])
                return tp

            # ---- preamble (all DMA issues hoisted before compute) ----
            hT_sb = None
            with tc.tile_pool(name="pre", bufs=1) as pre, \
                 tc.tile_pool(name="pmm", bufs=2, space="PSUM") as pmm:
                q_sb = pre.tile([128, 4, BL], bf16)
                wh_sb = pre.tile([128, 4, 512], bf16)
                wc_sb = pre.tile([128, 4, 512], bf16)
                y_sb = pre.tile([128, 3, 256], bf16)
                wy_sb = pre.tile([128, 3, 2048], bf16)
                lT_sb = pre.tile([128, 8, BL * R], bf16)
                wat_sb = pre.tile([128, 8, 512], bf16)
                wua_sb = pre.tile([128, 8, 512], bf16)
                g_sb = pre.tile([128, 17, BL], bf16)
                wg_sb = pre.tile([128, 17, 512], bf16)
                for k in range(4):
                    dma(out=q_sb[:, k, :], in_=qT_d[128 * k:128 * (k + 1), :])
                    dma(out=wh_sb[:, k, :], in_=whq_d[128 * k:128 * (k + 1), :])
                    dma(out=wc_sb[:, k, :], in_=wcq_d[128 * k:128 * (k + 1), :])
                for k in range(2):
                    dma(out=y_sb[:, k, :], in_=yT_d[128 * k:128 * (k + 1), :])
                    dma(out=wy_sb[:, k, :], in_=wyq_d[128 * k:128 * (k + 1), :])
                dma(out=y_sb[0:1, 2, :], in_=yT_d[256:257, :])
                dma(out=wy_sb[0:1, 2, :], in_=wyq_d[256:257, :])
                for k in range(8):
                    dma(out=lT_sb[:, k, :], in_=lT_d[128 * k:128 * (k + 1), :])
                    dma(out=wat_sb[:, k, :], in_=watq_d[128 * k:128 * (k + 1), :])
                    dma(out=wua_sb[:, k, :], in_=wuaq_d[128 * k:128 * (k + 1), :])
                for k in range(16):
                    dma(out=g_sb[:, k, :], in_=gT_d[128 * k:128 * (k + 1), :])
                    dma(out=wg_sb[:, k, :], in_=wgq_d[128 * k:128 * (k + 1), :])
                dma(out=g_sb[0:1, 16, :], in_=gT_d[2048:2049, :])
                dma(out=wg_sb[0:1, 16, :], in_=wgq_d[2048:2049, :])

                # h0 / c0 (quad)
                hq0 = pmm.tile([128, 128], f32, name="hq0", tag="mmq")
                cq0 = pmm.tile([128, 128], f32, name="cq0", tag="mmq")
                for dst, w_sb in ((hq0, wh_sb), (cq0, wc_sb)):
                    for k in range(4):
                        for j in range(4):
                            quad_mm(dst, q_sb[:, k, :],
                                    w_sb[:, k, 128 * j:128 * (j + 1)], j,
                                    start=(k == 0), stop=(k == 3))
                h_sb = stb.tile([128, 128], bf16, name="h", tag="h")
                vcp(h_sb[0:P104, :], hq0[0:P104, :])
                vcp(cq_sb[0:P104, :], cq0[0:P104, :])
                tp = transpose104(h_sb[0:P104, :])
                hT_sb = stb.tile([128, 4, 32], bf16, name="hT", tag="hT")
                vcp(hT_sb[:, :, 0:BL], tp[:, :, 0:BL])

                # gates_y -> DRAM scratch (bf16, quad-permuted cols)
                for m in range(2):
                    for nb in range(4):
                        ps = pmm.tile([128, 512], f32, name="gyps", tag="mmq")
                        ns = slice(512 * nb, 512 * (nb + 1))
                        for k in range(3):
                            lhsT = (y_sb[0:1, 2, 128 * m:128 * (m + 1)] if k == 2
                                    else y_sb[:, k, 128 * m:128 * (m + 1)])
                            rhs = (wy_sb[0:1, 2, ns] if k == 2
                                   else wy_sb[:, k, ns])
                            mm(ps[:, :], lhsT, rhs, start=(k == 0), stop=(k == 2))
                        lg = stb.tile([128, 512], bf16, name="lg", tag="lg")
                        vcp(lg[:, :], ps[:, :])
                        dma(out=gy_dram[128 * m:128 * (m + 1), ns], in_=lg[:, :])

                # apT + pbd (shared lT)
                for hk in range(4):
                    ps = pmm.tile([128, BL * R], f32, name="apps", tag="mmq")
                    for k in range(8):
                        mm(ps[:, :], wat_sb[:, k, 128 * hk:128 * (hk + 1)],
                           lT_sb[:, k, :], start=(k == 0), stop=(k == 7))
                    vcp(apT_sb[:, hk, :], ps[:, :])
                for mt in range(4):
                    ps = pmm.tile([128, 512], f32, name="pbps", tag="mmq")
                    for k in range(8):
                        mm(ps[0:2 * R, :],
                           lT_sb[:, k, 2 * R * mt:2 * R * (mt + 1)],
                           wua_sb[:, k, :], start=(k == 0), stop=(k == 7))
                    vcp(pbd_sb[0:2 * R, mt, :], ps[0:2 * R, :])

                # o0 (plain [8,512] then 4 narrow transposes into otb slot 0)
                o0ps = pmm.tile([BL, 512], f32, name="o0ps", tag="mmq")
                for k in range(17):
                    lhsT = g_sb[0:1, 16, :] if k == 16 else g_sb[:, k, :]
                    rhs = wg_sb[0:1, 16, :] if k == 16 else wg_sb[:, k, :]
                    mm(o0ps[:, :], lhsT, rhs, start=(k == 0), stop=(k == 16))
                o0_sb = stb.tile([BL, 512], bf16, name="o0", tag="o0")
                vcp(o0_sb[:, :], o0ps[:, :])
                tp0 = ptp.tile([128, 4, 32], bf16, name="tp0", tag="tp")
                for k in range(4):
                    nc.tensor.transpose(tp0[:, k, :],
                                        o0_sb[:, 128 * k:128 * (k + 1)],
                                        idb_sb[0:8, 0:32])
                vcp(otb_sb[:, :, 0, :], tp0[:, :, 0:BL])

                # gates(0)
                gytq = gyp.tile([BL, 2048], bf16, name="gytq", tag="gytq")
                dma(out=gytq[:, :], in_=gy_dram[0:BL, :])
                gq = pg.tile([128, 512], f32, name="gq", tag="gq")
                for j in range(4):
                    quad_mm(gq, id8, gytq[0:8, 512 * j:512 * (j + 1)], j,
                            start=True, stop=False)
                for k in range(4):
                    for j in range(4):
                        quad_mm(gq, hT_sb[:, k, 0:BL],
                                wr_sb[:, 4 + k, 512 * j:512 * (j + 1)], j,
                                start=False, stop=False)
                for k in range(4):
                    for j in range(4):
                        quad_mm(gq, otb_sb[:, k, 0, :],
                                wr_sb[:, k, 512 * j:512 * (j + 1)], j,
                                start=False, stop=(k == 3))

            # vocab weight loads: 2 issued per step over the first 16 steps
            wv_pairs = [(vc, k) for vc in range(8) for k in range(4)]

            pe1 = stack.enter_context(tc.tile_pool(name="pe1", bufs=1, space="PSUM"))
            pv = stack.enter_context(tc.tile_pool(name="pv", bufs=1, space="PSUM"))
            pvoc = stack.enter_context(tc.tile_pool(name="pvoc", bufs=2, space="PSUM"))

            # vocab units: (m_tile, col_base, n_size); M-tile 0 interleaved
            nmm = [(0, 512), (512, 512), (1024, 226)]
            units = [(m, 1250 * vc + nb, nsz)
                     for m in range(2) for vc in range(8) for nb, nsz in nmm]

            def vocab_unit(m, cb, nsz):
                ps = pvoc.tile([128, 512], f32, name="vps", tag="vps")
                for k in range(4):
                    mm(ps[:, 0:nsz], otb_sb[:, k, 1 + 16 * m:17 + 16 * m, :],
                       wv_sb[:, k, cb:cb + nsz], start=(k == 0), stop=(k == 3),
                       skip_group_check=True)
                lg = stb.tile([128, 512], bf16, name="lg", tag="lg")
                nc.scalar.copy(lg[:, 0:nsz], ps[:, 0:nsz])
                dma(out=out_d[128 * m:128 * (m + 1), cb:cb + nsz],
                    in_=lg[:, 0:nsz])

            # ---- recurrence ----
            for t in range(T):
                if t < 16:
                    for vc, k in wv_pairs[2 * t:2 * t + 2]:
                        vs = slice(1250 * vc, 1250 * (vc + 1))
                        dma(out=wv_sb[:, k, vs],
                            in_=wv_d[128 * k:128 * (k + 1), vs])
                if t < T - 1:
                    gytq = gyp.tile([BL, 2048], bf16, name="gytq", tag="gytq")
                    dma(out=gytq[:, :],
                        in_=gy_dram[BL * (t + 1):BL * (t + 2), :])

                # LSTM cell on quad layout; gq cols per group: [g|i|f|og]*128
                tgall = stb.tile([128, 512], bf16, name="tgall", tag="tgall")
                nc.scalar.activation(tgall[0:P104, 0:256], gq[0:P104, 0:256],
                                     AF.Tanh)
                nc.scalar.activation(tgall[0:P104, 256:512],
                                     gq[0:P104, 256:512], AF.Tanh)
                tg = tgall[0:P104, 0:128]
                ti = tgall[0:P104, 128:256]
                tf_ = tgall[0:P104, 256:384]
                tog = tgall[0:P104, 384:512]
                t1 = stb.tile([128, 128], f32, name="t1", tag="t1")
                stt(t1[0:P104, :], ti, 1.0, tg, op0=ADD, op1=MULT)
                m2 = stb.tile([128, 128], f32, name="m2", tag="m2")
                stt(m2[0:P104, :], tf_, 1.0, cq_sb[0:P104, :], op0=ADD, op1=MULT)
                stt(cq_sb[0:P104, :], m2[0:P104, :], 0.5, t1[0:P104, :],
                    op0=MULT, op1=ADD)
                tc2 = stb.tile([128, 128], bf16, name="tc2", tag="tc2")
                nc.scalar.activation(tc2[0:P104, :], cq_sb[0:P104, :], AF.Tanh,
                                     scale=0.5)
                h_sb = stb.tile([128, 128], bf16, name="h", tag="h")
                stt(h_sb[0:P104, :], tog, 1.0, tc2[0:P104, :], op0=ADD, op1=MULT)

                tp = transpose104(h_sb[0:P104, :])
                hT_sb = stb.tile([128, 4, 32], bf16, name="hT", tag="hT")
                vcp(hT_sb[:, :, 0:BL], tp[:, :, 0:BL])

                # attention E first (critical path), masked via id8 wave
                eq = pe1.tile([128, 2 * R + 1], f32, name="eq", tag="eq")
                for j in range(4):
                    quad_mm(eq, id8, mask_sb[0:8, j, :], j, start=True,
                            stop=False)
                for k in range(4):
                    for j in range(4):
                        mm(eq[32 * j:32 * j + BL, 0:2 * R], hT_sb[:, k, 0:BL],
                           apT_sb[:, k, 2 * R * j:2 * R * (j + 1)],
                           start=False, stop=(k == 3), skip_group_check=True,
                           tile_position=(0, 32 * j))

                # PE filler sized to the exp-chain: gates(t+1) gy + h-part
                if t < T - 1:
                    gq_next = pg.tile([128, 512], f32, name="gq", tag="gq")
                    for j in range(4):
                        quad_mm(gq_next, id8, gytq[0:8, 512 * j:512 * (j + 1)],
                                j, start=True, stop=False)
                    for k in range(4):
                        for j in range(4):
                            quad_mm(gq_next, hT_sb[:, k, 0:BL],
                                    wr_sb[:, 4 + k, 512 * j:512 * (j + 1)], j,
                                    start=False, stop=False)

                expq = stb.tile([128, 2 * R + 1], bf16, name="expq", tag="expq")
                ssum = stb.tile([128, 1], f32, name="ssum", tag="ssum")
                nc.scalar.activation(expq[0:P104, :], eq[0:P104, :], AF.Exp,
                                     accum_out=ssum[0:P104, :])
                rs = stb.tile([128, 1], f32, name="rs", tag="rs")
                nc.vector.reciprocal(rs[0:P104, :], ssum[0:P104, :])
                attq = stb.tile([128, 2 * R], bf16, name="attq", tag="attq")
                nc.vector.tensor_scalar_mul(attq[0:P104, :],
                                            expq[0:P104, 0:2 * R],
                                            rs[0:P104, :])
                tpE = transpose104(attq[0:P104, :])
                vcp(abd_sb[0:2 * R, :, 0:BL], tpE[0:2 * R, :, 0:BL])

                # vo += b_u + att-weighted P  (h-part already accumulated)
                for m in range(4):
                    kk = 2 * R + 1 if m == 0 else 2 * R
                    for j in range(4):
                        quad_mm(vq, abd_sb[0:kk, m, 0:BL],
                                pbd_sb[0:kk, m, 128 * j:128 * (j + 1)], j,
                                start=False, stop=(m == 3))
                o_sb = stb.tile([128, 128], bf16, name="o", tag="o")
                nc.scalar.activation(o_sb[0:P104, :], vq[0:P104, :], AF.Tanh)

                tpo = transpose104(o_sb[0:P104, :])
                vcp(otb_sb[:, :, t + 1, :], tpo[:, :, 0:BL])

                # gates(t+1): o-part
                if t < T - 1:
                    for k in range(4):
                        for j in range(4):
                            quad_mm(gq_next, otb_sb[:, k, t + 1, :],
                                    wr_sb[:, k, 512 * j:512 * (j + 1)], j,
                                    start=False, stop=(k == 3))
                    gq = gq_next

                # second vocab unit fills the next cell's PE-idle window
                if t >= 24:
                    vocab_unit(*units[16 + (t - 24)])

            # ---- vocab M-tile 1 ----
            for u in range(24, 48):
                vocab_unit(*units[u])

    nc.compile()
    return nc


_STATE = {}


def kernel(**inputs):
    from concourse.bass_utils import run_bass_kernel_spmd

    in_maps = prep_inputs(**inputs)
    if "nc" not in _STATE:
        _STATE["nc"] = build_nc()
    nc = _STATE["nc"]
    res = run_bass_kernel_spmd(nc, in_maps, core_ids=list(range(NCORES)))
    bv = np.asarray(inputs["b_vocab"], np.float32)
    full = np.empty((B, T, VOCAB), np.float32)
    for c in range(NCORES):
        o = res.results[c]["out"].astype(np.float32) + bv[None, :]
        full[c * BL:(c + 1) * BL] = o.reshape(T, BL, VOCAB).transpose(1, 0, 2)
    return full


# revision 20
# speedup vs baseline: 1.1764x; 1.1764x over previous
"""AnswerDecoder (LSTM decoder w/ visual attention) on 8 TRN2 NeuronCores.

Strategy: pure data-parallel over batch (8 batches/core), zero collectives.

v2 design notes (all relative to the measured v1 trace: 787us, PE-bound):
- "Quad" layout: partition 32j+b holds (hidden-slice j, batch b). All LSTM
  cell elementwise work runs as single [104, N] instructions instead of 4-8
  [8, N] ones (ACT/DVE cost is free-dim-dominated).
- 4-way PE column tiling: the four quad groups' matmuls use tile_position
  (0, 32j) and stream concurrently (measured 82.5 ns/MM for N=512 bf16 vs
  231.8 serial). Same-bank disjoint-partition accumulation verified OK.
- bf16 weights/activations everywhere on the PE (f32r at N<256 runs at 1/4
  rate; bf16 is 1 cycle/row always). c-state and PSUM stay f32.
- One batched PE transpose per h/att/o per step ([104,128] -> [128,128] via
  zero-padded identity) instead of 4 narrow transposes each.
- Softmax without max-subtraction (|e| << 80 so fp32 exp is safe); the
  block-diagonal mask is folded into the e-matmul as a K=8 identity wave;
  row sums come free from the Exp accumulator; normalization happens on the
  exp output before transposing.
- W_u bias enters via a ones-row in abd x b_u row in pbd; vocab bias is
  added on the host.
- Vocab projection: M-tile 0 (steps 0-15) is interleaved into PE idle slots
  of steps 16-31; only M-tile 1 runs after the loop. Output is bf16
  (upcast on host).
"""

import numpy as np

B, T, R = 64, 32, 49
LOCAL, QVEC, EMB, HID, VOCAB = 1024, 512, 256, 512, 10000
START_IDX = 1
NCORES = 8
BL = B // NCORES        # 8 batches per core
ROWS = T * BL           # 256 output rows per core, t-major (row = t*BL + b)
NEG = -60000.0          # mask value; exp(NEG + e) underflows to 0 in fp32


def _quad_perm_scale():
    """Device gate-column order: group j, gate [g,i,f,og], offset f.
    Returns (perm, scale): device col -> ref 4H row, and the 0.5 tanh-half
    scaling for i/f/og. Order [g,i,...] lets the cell tanh split into two
    contiguous 256-col ops with (g,i) first so t1 starts early."""
    # ref row ranges: i 0:512, f 512:1024, g 1024:1536, o 1536:2048
    base = {0: 1024, 1: 0, 2: 512, 3: 1536}      # device gate idx -> ref base
    perm = np.empty(2048, np.int64)
    scale = np.empty(2048, np.float32)
    for j in range(4):
        for g in range(4):
            cols = slice(j * 512 + g * 128, j * 512 + g * 128 + 128)
            perm[cols] = base[g] + 128 * j + np.arange(128)
            scale[cols] = 1.0 if g == 0 else 0.5
    return perm, scale


def prep_inputs(image_local_features, image_global_features, question_vectors,
                answers, emb, W_g2o, b_g2o, W_h, W_c, W_ih, W_hh, b_ih, b_hh,
                W_attn, W_u, b_u, W_vocab, b_vocab):
    """Host-side data layout prep. Returns list of per-core input dicts."""
    import ml_dtypes
    bf16 = ml_dtypes.bfloat16
    f32 = np.float32

    L = np.asarray(image_local_features, f32)                   # [B,R,F]
    g = np.asarray(image_global_features, f32)                  # [B,2F]
    q = np.asarray(question_vectors, f32)                       # [B,Q]
    ans = np.asarray(answers).astype(np.int64)                  # [B,T]
    emb = np.asarray(emb, f32)

    perm, qscale = _quad_perm_scale()
    # recurrent weights: K = [o (512); h (512)], h-part halved (h' = 2h)
    W_cat = np.concatenate([W_ih[:, EMB:EMB + HID], 0.5 * np.asarray(W_hh, f32)],
                           axis=1)                              # [2048, 1024]
    wrq = (W_cat.T[:, perm] * qscale[None, :]).astype(bf16)     # [1024, 2048]
    wy_full = np.concatenate([np.asarray(W_ih, f32)[:, :EMB].T,
                              (np.asarray(b_ih, f32) + np.asarray(b_hh, f32))[None, :]],
                             axis=0)                            # [257, 2048ref]
    wyq = (wy_full[:, perm] * qscale[None, :]).astype(bf16)     # [257, 2048]

    whq = (2.0 * np.asarray(W_h, f32).T).astype(bf16)           # [512, 512]
    wcq = (2.0 * np.asarray(W_c, f32).T).astype(bf16)           # [512, 512]
    wgq = np.concatenate([np.asarray(W_g2o, f32).T,
                          np.asarray(b_g2o, f32)[None, :]], 0).astype(bf16)  # [2049,512]
    watq = (0.5 * np.asarray(W_attn, f32).T).astype(bf16)       # [1024, 512]
    wuaq = np.ascontiguousarray(np.asarray(W_u, f32)[:, :LOCAL].T).astype(bf16)
    wuhq = (0.5 * np.asarray(W_u, f32)[:, LOCAL:].T).astype(bf16)  # [512, 512]
    buq = np.asarray(b_u, f32)[None, :].astype(bf16)            # [1, 512]
    wv = np.ascontiguousarray(np.asarray(W_vocab, f32).T).astype(bf16)  # [512,10000]

    # col 2R is an epsilon column (-55 -> exp ~ 1.3e-24): keeps every row's
    # exp-sum nonzero so 1/ssum stays finite on fully-masked (off-diagonal)
    # rows; abd only consumes cols 0:2R so it never reaches vo.
    maskq = np.full((BL, 4, 2 * R + 1), NEG, f32)
    maskq[:, :, 2 * R] = -55.0
    for j in range(4):
        maskq[2 * j, j, 0:R] = 0.0
        maskq[2 * j + 1, j, R:2 * R] = 0.0
    maskq = maskq.astype(bf16)
    idb = np.eye(128, dtype=f32).astype(bf16)
    ones8 = np.ones((1, BL), f32).astype(bf16)

    # teacher-forced input embeddings: y_seq[t] = emb[ans[:, t-1]], y_seq[0]=emb[1]
    idx = np.concatenate([np.full((B, 1), START_IDX, np.int64), ans[:, :-1]], 1)
    y_emb = emb[idx]                                            # [B,T,EMB]

    def chunked(a, rows=128):
        """[K, N] -> [128, K//128, N] (device layout, partition-major),
        zero-padding K up to a multiple of 128."""
        a = np.asarray(a)
        k, n = a.shape
        nch = -(-k // rows)
        if k < nch * rows:
            pad = np.zeros((nch * rows - k, n), a.dtype)
            a = np.concatenate([a, pad], 0)
        return np.ascontiguousarray(
            a.reshape(nch, rows, n).transpose(1, 0, 2))

    shared = {
        "wyq": chunked(wyq), "wrq": chunked(wrq), "whq": chunked(whq),
        "wcq": chunked(wcq), "wgq": chunked(wgq), "watq": chunked(watq),
        "wuaq": chunked(wuaq), "wuhq": chunked(wuhq), "buq": buq,
        "wv": chunked(wv), "maskq": maskq, "idb": idb, "ones8": ones8,
    }
    in_maps = []
    for c in range(NCORES):
        s = slice(c * BL, (c + 1) * BL)
        ye = y_emb[s].transpose(1, 0, 2).reshape(ROWS, EMB)     # [(t,b), EMB]
        yT = np.concatenate([ye.T, np.ones((1, ROWS), f32)], 0).astype(bf16)
        lT = np.ascontiguousarray(L[s].reshape(BL * R, LOCAL).T).astype(bf16)
        qT = np.ascontiguousarray(q[s].T).astype(bf16)          # [512,8]
        gT = np.concatenate([g[s].T, np.ones((1, BL), f32)], 0).astype(bf16)
        d = {"yT": chunked(yT), "lT": chunked(lT), "qT": chunked(qT),
             "gT": chunked(gT)}
        d.update(shared)
        in_maps.append(d)
    return in_maps


def build_nc():
    """Build the per-core Bass graph (identical on all 8 cores)."""
    from concourse import bacc, tile
    import concourse.mybir as mybir

    f32 = mybir.dt.float32
    bf16 = mybir.dt.bfloat16
    AF = mybir.ActivationFunctionType
    ALU = mybir.AluOpType

    nc = bacc.Bacc("TRN2", target_bir_lowering=False, debug=False,
                   num_devices=NCORES)

    def dparam(name, shape, dt=bf16):
        return nc.dram_tensor(name, list(shape), dt, kind="ExternalInput").ap()

    yT_d = dparam("yT", [257, 256])
    wyq_d = dparam("wyq", [257, 2048])
    wrq_d = dparam("wrq", [1024, 2048])
    lT_d = dparam("lT", [1024, BL * R])
    watq_d = dparam("watq", [1024, 512])
    wuaq_d = dparam("wuaq", [1024, 512])
    wuhq_d = dparam("wuhq", [512, 512])
    qT_d = dparam("qT", [512, BL])
    whq_d = dparam("whq", [512, 512])
    wcq_d = dparam("wcq", [512, 512])
    gT_d = dparam("gT", [2049, BL])
    wgq_d = dparam("wgq", [2049, 512])
    buq_d = dparam("buq", [1, 512])
    wv_d = dparam("wv", [512, VOCAB])
    mask_d = dparam("maskq", [BL, 4, 2 * R + 1])
    idb_d = dparam("idb", [128, 128])
    ones8_d = dparam("ones8", [1, BL])
    out_d = nc.dram_tensor("out", [ROWS, VOCAB], bf16, kind="ExternalOutput").ap()

    mm = nc.tensor.matmul
    ADD, MULT = ALU.add, ALU.mult
    stt = nc.vector.scalar_tensor_tensor
    vcp = nc.vector.tensor_copy
    P104 = 104  # 3*32 + 8: spans all four quad groups

    with tile.TileContext(nc) as tc:
        import contextlib
        stack = contextlib.ExitStack()
        with stack:
            pers = stack.enter_context(tc.tile_pool(name="pers", bufs=1))
            stb = stack.enter_context(tc.tile_pool(name="stb", bufs=2))
            gyp = stack.enter_context(tc.tile_pool(name="gyp", bufs=2))
            gyd = stack.enter_context(tc.tile_pool(name="gyd", bufs=1, space="DRAM"))
            pg = stack.enter_context(tc.tile_pool(name="pg", bufs=2, space="PSUM"))
            ptp = stack.enter_context(tc.tile_pool(name="ptp", bufs=2, space="PSUM"))

            dma = nc.sync.dma_start

            # ---- persistent SBUF ----
            wr_sb = pers.tile([128, 8, 2048], bf16)
            wuh_sb = pers.tile([128, 4, 512], bf16)
            apT_sb = pers.tile([128, 4, BL * R], bf16)
            pbd_sb = pers.tile([128, 4, 512], bf16)
            otb_sb = pers.tile([128, 4, T + 1, BL], bf16)
            wv_sb = pers.tile([128, 4, VOCAB], bf16)
            cq_sb = pers.tile([128, 128], f32)
            abd_sb = pers.tile([128, 4, 32], bf16)
            mask_sb = pers.tile([BL, 4, 2 * R + 1], bf16)
            idb_sb = pers.tile([128, 128], bf16)
            gy_dram = gyd.tile([ROWS, 2048], bf16)

            dma(out=mask_sb[:, :, :], in_=mask_d[:, :, :])
            dma(out=idb_sb[:, :], in_=idb_d[:, :])
            dma(out=abd_sb[2 * R:2 * R + 1, 0, 0:BL], in_=ones8_d[:, :])
            dma(out=pbd_sb[2 * R:2 * R + 1, 0, :], in_=buq_d[:, :])
            for k in range(4):
                dma(out=wuh_sb[:, k, :], in_=wuhq_d[128 * k:128 * (k + 1), :])
            for k in range(8):
                dma(out=wr_sb[:, k, :], in_=wrq_d[128 * k:128 * (k + 1), :])

            id8 = idb_sb[0:8, 0:8]
            idT = idb_sb[0:P104, 0:128]   # zero-padded transpose identity

            def quad_mm(out_t, lhsT, rhs, j, start, stop):
                mm(out_t[32 * j:32 * j + BL, :], lhsT, rhs, start=start,
                   stop=stop, skip_group_check=True, tile_position=(0, 32 * j))

            def transpose104(in_ap, src_pool=None):
                """[104, F] -> ptp tile [128, 4, 32] (cols 32k+b valid)."""
                tp = ptp.tile([128, 4, 32], bf16, name="tp", tag="tp")
                nP = in_ap.shape[0]
                nc.tensor.transpose(tp[0:in_ap.shape[1], :, :], in_ap,
                                    idb_sb[0:nP, 0:128])
                return tp

            # ---- preamble (all DMA issues hoisted before compute) ----
            hT_sb = None
            with tc.tile_pool(name="pre", bufs=1) as pre, \
                 tc.tile_pool(name="pmm", bufs=2, space="PSUM") as pmm:
                q_sb = pre.tile([128, 4, BL], bf16)
                wh_sb = pre.tile([128, 4, 512], bf16)
                wc_sb = pre.tile([128, 4, 512], bf16)
                y_sb = pre.tile([128, 3, 256], bf16)
                wy_sb = pre.tile([128, 3, 2048], bf16)
                lT_sb = pre.tile([128, 8, BL * R], bf16)
                wat_sb = pre.tile([128, 8, 512], bf16)
                wua_sb = pre.tile([128, 8, 512], bf16)
                g_sb = pre.tile([128, 17, BL], bf16)
                wg_sb = pre.tile([128, 17, 512], bf16)
                for k in range(4):
                    dma(out=q_sb[:, k, :], in_=qT_d[128 * k:128 * (k + 1), :])
                    dma(out=wh_sb[:, k, :], in_=whq_d[128 * k:128 * (k + 1), :])
                    dma(out=wc_sb[:, k, :], in_=wcq_d[128 * k:128 * (k + 1), :])
                for k in range(2):
                    dma(out=y_sb[:, k, :], in_=yT_d[128 * k:128 * (k + 1), :])
                    dma(out=wy_sb[:, k, :], in_=wyq_d[128 * k:128 * (k + 1), :])
                dma(out=y_sb[0:1, 2, :], in_=yT_d[256:257, :])
                dma(out=wy_sb[0:1, 2, :], in_=wyq_d[256:257, :])
                for k in range(8):
                    dma(out=lT_sb[:, k, :], in_=lT_d[128 * k:128 * (k + 1), :])
                    dma(out=wat_sb[:, k, :], in_=watq_d[128 * k:128 * (k + 1), :])
                    dma(out=wua_sb[:, k, :], in_=wuaq_d[128 * k:128 * (k + 1), :])
                for k in range(16):
                    dma(out=g_sb[:, k, :], in_=gT_d[128 * k:128 * (k + 1), :])
                    dma(out=wg_sb[:, k, :], in_=wgq_d[128 * k:128 * (k + 1), :])
                dma(out=g_sb[0:1, 16, :], in_=gT_d[2048:2049, :])
                dma(out=wg_sb[0:1, 16, :], in_=wgq_d[2048:2049, :])

                # h0 / c0 (quad)
                hq0 = pmm.tile([128, 128], f32, name="hq0", tag="mmq")
                cq0 = pmm.tile([128, 128], f32, name="cq0", tag="mmq")
                for dst, w_sb in ((hq0, wh_sb), (cq0, wc_sb)):
                    for k in range(4):
                        for j in range(4):
                            quad_mm(dst, q_sb[:, k, :],
                                    w_sb[:, k, 128 * j:128 * (j + 1)], j,
                                    start=(k == 0), stop=(k == 3))
                h_sb = stb.tile([128, 128], bf16, name="h", tag="h")
                vcp(h_sb[0:P104, :], hq0[0:P104, :])
                vcp(cq_sb[0:P104, :], cq0[0:P104, :])
                tp = transpose104(h_sb[0:P104, :])
                hT_sb = stb.tile([128, 4, 32], bf16, name="hT", tag="hT")
                vcp(hT_sb[:, :, 0:BL], tp[:, :, 0:BL])

                # gates_y -> DRAM scratch (bf16, quad-permuted cols)
                for m in range(2):
                    for nb in range(4):
                        ps = pmm.tile([128, 512], f32, name="gyps", tag="mmq")
                        ns = slice(512 * nb, 512 * (nb + 1))
                        for k in range(3):
                            lhsT = (y_sb[0:1, 2, 128 * m:128 * (m + 1)] if k == 2
                                    else y_sb[:, k, 128 * m:128 * (m + 1)])
                            rhs = (wy_sb[0:1, 2, ns] if k == 2
                                   else wy_sb[:, k, ns])
                            mm(ps[:, :], lhsT, rhs, start=(k == 0), stop=(k == 2))
                        lg = stb.tile([128, 512], bf16, name="lg", tag="lg")
                        vcp(lg[:, :], ps[:, :])
                        dma(out=gy_dram[128 * m:128 * (m + 1), ns], in_=lg[:, :])

                # apT + pbd (shared lT)
                for hk in range(4):
                    ps = pmm.tile([128, BL * R], f32, name="apps", tag="mmq")
                    for k in range(8):
                        mm(ps[:, :], wat_sb[:, k, 128 * hk:128 * (hk + 1)],
                           lT_sb[:, k, :], start=(k == 0), stop=(k == 7))
                    vcp(apT_sb[:, hk, :], ps[:, :])
                for mt in range(4):
                    ps = pmm.tile([128, 512], f32, name="pbps", tag="mmq")
                    for k in range(8):
                        mm(ps[0:2 * R, :],
                           lT_sb[:, k, 2 * R * mt:2 * R * (mt + 1)],
                           wua_sb[:, k, :], start=(k == 0), stop=(k == 7))
                    vcp(pbd_sb[0:2 * R, mt, :], ps[0:2 * R, :])

                # o0 (plain [8,512] then 4 narrow transposes into otb slot 0)
                o0ps = pmm.tile([BL, 512], f32, name="o0ps", tag="mmq")
                for k in range(17):
                    lhsT = g_sb[0:1, 16, :] if k == 16 else g_sb[:, k, :]
                    rhs = wg_sb[0:1, 16, :] if k == 16 else wg_sb[:, k, :]
                    mm(o0ps[:, :], lhsT, rhs, start=(k == 0), stop=(k == 16))
                o0_sb = stb.tile([BL, 512], bf16, name="o0", tag="o0")
                vcp(o0_sb[:, :], o0ps[:, :])
                tp0 = ptp.tile([128, 4, 32], bf16, name="tp0", tag="tp")
                for k in range(4):
                    nc.tensor.transpose(tp0[:, k, :],
                                        o0_sb[:, 128 * k:128 * (k + 1)],
                                        idb_sb[0:8, 0:32])
                vcp(otb_sb[:, :, 0, :], tp0[:, :, 0:BL])

                # gates(0)
                gytq = gyp.tile([BL, 2048], bf16, name="gytq", tag="gytq")
                dma(out=gytq[:, :], in_=gy_dram[0:BL, :])
                gq = pg.tile([128, 512], f32, name="gq", tag="gq")
                for j in range(4):
                    quad_mm(gq, id8, gytq[0:8, 512 * j:512 * (j + 1)], j,
                            start=True, stop=False)
                for k in range(4):
                    for j in range(4):
                        quad_mm(gq, hT_sb[:, k, 0:BL],
                                wr_sb[:, 4 + k, 512 * j:512 * (j + 1)], j,
                                start=False, stop=False)
                for k in range(4):
                    for j in range(4):
                        quad_mm(gq, otb_sb[:, k, 0, :],
                                wr_sb[:, k, 512 * j:512 * (j + 1)], j,
                                start=False, stop=(k == 3))

            # vocab weight loads: 2 issued per step over the first 16 steps
            wv_pairs = [(vc, k) for vc in range(8) for k in range(4)]

            pe1 = stack.enter_context(tc.tile_pool(name="pe1", bufs=1, space="PSUM"))
            pv = stack.enter_context(tc.tile_pool(name="pv", bufs=1, space="PSUM"))
            pvoc = stack.enter_context(tc.tile_pool(name="pvoc", bufs=2, space="PSUM"))

            # vocab units: (m_tile, col_base, n_size); M-tile 0 interleaved
            nmm = [(0, 512), (512, 512), (1024, 226)]
            units = [(m, 1250 * vc + nb, nsz)
                     for m in range(2) for vc in range(8) for nb, nsz in nmm]

            def vocab_unit(m, cb, nsz):
                ps = pvoc.tile([128, 512], f32, name="vps", tag="vps")
                for k in range(4):
                    mm(ps[:, 0:nsz], otb_sb[:, k, 1 + 16 * m:17 + 16 * m, :],
                       wv_sb[:, k, cb:cb + nsz], start=(k == 0), stop=(k == 3),
                       skip_group_check=True)
                lg = stb.tile([128, 512], bf16, name="lg", tag="lg")
                nc.scalar.copy(lg[:, 0:nsz], ps[:, 0:nsz])
                dma(out=out_d[128 * m:128 * (m + 1), cb:cb + nsz],
                    in_=lg[:, 0:nsz])

            # ---- recurrence ----
            next_unit = 0
            for t in range(T):
                if t < 16:
                    for vc, k in wv_pairs[2 * t:2 * t + 2]:
                        vs = slice(1250 * vc, 1250 * (vc + 1))
                        dma(out=wv_sb[:, k, vs],
                            in_=wv_d[128 * k:128 * (k + 1), vs])
                if t < T - 1:
                    gytq = gyp.tile([BL, 2048], bf16, name="gytq", tag="gytq")
                    dma(out=gytq[:, :],
                        in_=gy_dram[BL * (t + 1):BL * (t + 2), :])

                # LSTM cell on quad layout; gq cols per group: [g|i|f|og]*128
                tgall = stb.tile([128, 512], bf16, name="tgall", tag="tgall")
                nc.scalar.activation(tgall[0:P104, 0:256], gq[0:P104, 0:256],
                                     AF.Tanh)
                nc.scalar.activation(tgall[0:P104, 256:512],
                                     gq[0:P104, 256:512], AF.Tanh)
                tg = tgall[0:P104, 0:128]
                ti = tgall[0:P104, 128:256]
                tf_ = tgall[0:P104, 256:384]
                tog = tgall[0:P104, 384:512]
                t1 = stb.tile([128, 128], f32, name="t1", tag="t1")
                stt(t1[0:P104, :], ti, 1.0, tg, op0=ADD, op1=MULT)
                m2 = stb.tile([128, 128], f32, name="m2", tag="m2")
                stt(m2[0:P104, :], tf_, 1.0, cq_sb[0:P104, :], op0=ADD, op1=MULT)
                stt(cq_sb[0:P104, :], m2[0:P104, :], 0.5, t1[0:P104, :],
                    op0=MULT, op1=ADD)
                tc2 = stb.tile([128, 128], bf16, name="tc2", tag="tc2")
                nc.scalar.activation(tc2[0:P104, :], cq_sb[0:P104, :], AF.Tanh,
                                     scale=0.5)
                h_sb = stb.tile([128, 128], bf16, name="h", tag="h")
                stt(h_sb[0:P104, :], tog, 1.0, tc2[0:P104, :], op0=ADD, op1=MULT)

                tp = transpose104(h_sb[0:P104, :])
                hT_sb = stb.tile([128, 4, 32], bf16, name="hT", tag="hT")
                vcp(hT_sb[:, :, 0:BL], tp[:, :, 0:BL])

                # attention E first (critical path), masked via id8 wave
                eq = pe1.tile([128, 2 * R + 1], f32, name="eq", tag="eq")
                for j in range(4):
                    quad_mm(eq, id8, mask_sb[0:8, j, :], j, start=True,
                            stop=False)
                for k in range(4):
                    for j in range(4):
                        mm(eq[32 * j:32 * j + BL, 0:2 * R], hT_sb[:, k, 0:BL],
                           apT_sb[:, k, 2 * R * j:2 * R * (j + 1)],
                           start=False, stop=(k == 3), skip_group_check=True,
                           tile_position=(0, 32 * j))

                # PE filler sized to the exp-chain: gates(t+1) gy + h-part
                if t < T - 1:
                    gq_next = pg.tile([128, 512], f32, name="gq", tag="gq")
                    for j in range(4):
                        quad_mm(gq_next, id8, gytq[0:8, 512 * j:512 * (j + 1)],
                                j, start=True, stop=False)
                    for k in range(4):
                        for j in range(4):
                            quad_mm(gq_next, hT_sb[:, k, 0:BL],
                                    wr_sb[:, 4 + k, 512 * j:512 * (j + 1)], j,
                                    start=False, stop=False)

                expq = stb.tile([128, 2 * R + 1], bf16, name="expq", tag="expq")
                ssum = stb.tile([128, 1], f32, name="ssum", tag="ssum")
                nc.scalar.activation(expq[0:P104, :], eq[0:P104, :], AF.Exp,
                                     accum_out=ssum[0:P104, :])
                rs = stb.tile([128, 1], f32, name="rs", tag="rs")
                nc.vector.reciprocal(rs[0:P104, :], ssum[0:P104, :])
                attq = stb.tile([128, 2 * R], bf16, name="attq", tag="attq")
                nc.vector.tensor_scalar_mul(attq[0:P104, :],
                                            expq[0:P104, 0:2 * R],
                                            rs[0:P104, :])
                tpE = transpose104(attq[0:P104, :])
                vcp(abd_sb[0:2 * R, :, 0:BL], tpE[0:2 * R, :, 0:BL])

                # vo: h-part runs right after attT (hides the abd copy),
                # then b_u + att-weighted P
                vq = pv.tile([128, 128], f32, name="vq", tag="vq")
                for k in range(4):
                    for j in range(4):
                        quad_mm(vq, hT_sb[:, k, 0:BL],
                                wuh_sb[:, k, 128 * j:128 * (j + 1)], j,
                                start=(k == 0), stop=False)
                for m in range(4):
                    kk = 2 * R + 1 if m == 0 else 2 * R
                    for j in range(4):
                        quad_mm(vq, abd_sb[0:kk, m, 0:BL],
                                pbd_sb[0:kk, m, 128 * j:128 * (j + 1)], j,
                                start=False, stop=(m == 3))
                o_sb = stb.tile([128, 128], bf16, name="o", tag="o")
                nc.scalar.activation(o_sb[0:P104, :], vq[0:P104, :], AF.Tanh)

                tpo = transpose104(o_sb[0:P104, :])
                vcp(otb_sb[:, :, t + 1, :], tpo[:, :, 0:BL])

                # gates(t+1): o-part
                if t < T - 1:
                    for k in range(4):
                        for j in range(4):
                            quad_mm(gq_next, otb_sb[:, k, t + 1, :],
                                    wr_sb[:, k, 512 * j:512 * (j + 1)], j,
                                    start=False, stop=(k == 3))
                    gq = gq_next

                # M-tile-0 vocab units fill the next cell's PE-idle window
                if t >= 16:
                    n_units = 2 if t >= 24 else 1
                    for _ in range(n_units):
                        vocab_unit(*units[next_unit])
                        next_unit += 1

            # ---- vocab M-tile 1 ----
            for u in range(next_unit, 48):
                vocab_unit(*units[u])

    nc.compile()
    return nc


_STATE = {}


def kernel(**inputs):
    from concourse.bass_utils import run_bass_kernel_spmd

    in_maps = prep_inputs(**inputs)
    if "nc" not in _STATE:
        _STATE["nc"] = build_nc()
    nc = _STATE["nc"]
    res = run_bass_kernel_spmd(nc, in_maps, core_ids=list(range(NCORES)))
    bv = np.asarray(inputs["b_vocab"], np.float32)
    full = np.empty((B, T, VOCAB), np.float32)
    for c in range(NCORES):
        o = res.results[c]["out"].astype(np.float32) + bv[None, :]
        full[c * BL:(c + 1) * BL] = o.reshape(T, BL, VOCAB).transpose(1, 0, 2)
    return full


# revision 26
# speedup vs baseline: 1.2568x; 1.0683x over previous
"""AnswerDecoder (LSTM decoder w/ visual attention) on 8 TRN2 NeuronCores.

Strategy: pure data-parallel over batch (8 batches/core), zero collectives.

v2 design notes (all relative to the measured v1 trace: 787us, PE-bound):
- "Quad" layout: partition 32j+b holds (hidden-slice j, batch b). All LSTM
  cell elementwise work runs as single [104, N] instructions instead of 4-8
  [8, N] ones (ACT/DVE cost is free-dim-dominated).
- 4-way PE column tiling: the four quad groups' matmuls use tile_position
  (0, 32j) and stream concurrently (measured 82.5 ns/MM for N=512 bf16 vs
  231.8 serial). Same-bank disjoint-partition accumulation verified OK.
- bf16 weights/activations everywhere on the PE (f32r at N<256 runs at 1/4
  rate; bf16 is 1 cycle/row always). c-state and PSUM stay f32.
- One batched PE transpose per h/att/o per step ([104,128] -> [128,128] via
  zero-padded identity) instead of 4 narrow transposes each.
- Softmax without max-subtraction (|e| << 80 so fp32 exp is safe); the
  block-diagonal mask is folded into the e-matmul as a K=8 identity wave;
  row sums come free from the Exp accumulator; normalization happens on the
  exp output before transposing.
- W_u bias enters via a ones-row in abd x b_u row in pbd; vocab bias is
  added on the host.
- Vocab projection: M-tile 0 (steps 0-15) is interleaved into PE idle slots
  of steps 16-31; only M-tile 1 runs after the loop. Output is bf16
  (upcast on host).
"""

import numpy as np

B, T, R = 64, 32, 49
LOCAL, QVEC, EMB, HID, VOCAB = 1024, 512, 256, 512, 10000
START_IDX = 1
NCORES = 8
BL = B // NCORES        # 8 batches per core
ROWS = T * BL           # 256 output rows per core, t-major (row = t*BL + b)
NEG = -60000.0          # mask value; exp(NEG + e) underflows to 0 in fp32


def _quad_perm_scale():
    """Device gate-column order: group j, gate [g,i,f,og], offset f.
    Returns (perm, scale): device col -> ref 4H row, and the 0.5 tanh-half
    scaling for i/f/og. Order [g,i,...] lets the cell tanh split into two
    contiguous 256-col ops with (g,i) first so t1 starts early."""
    # ref row ranges: i 0:512, f 512:1024, g 1024:1536, o 1536:2048
    base = {0: 1024, 1: 0, 2: 512, 3: 1536}      # device gate idx -> ref base
    perm = np.empty(2048, np.int64)
    scale = np.empty(2048, np.float32)
    for j in range(4):
        for g in range(4):
            cols = slice(j * 512 + g * 128, j * 512 + g * 128 + 128)
            perm[cols] = base[g] + 128 * j + np.arange(128)
            scale[cols] = 1.0 if g == 0 else 0.5
    return perm, scale


def prep_inputs(image_local_features, image_global_features, question_vectors,
                answers, emb, W_g2o, b_g2o, W_h, W_c, W_ih, W_hh, b_ih, b_hh,
                W_attn, W_u, b_u, W_vocab, b_vocab):
    """Host-side data layout prep. Returns list of per-core input dicts."""
    import ml_dtypes
    bf16 = ml_dtypes.bfloat16
    f32 = np.float32

    L = np.asarray(image_local_features, f32)                   # [B,R,F]
    g = np.asarray(image_global_features, f32)                  # [B,2F]
    q = np.asarray(question_vectors, f32)                       # [B,Q]
    ans = np.asarray(answers).astype(np.int64)                  # [B,T]
    emb = np.asarray(emb, f32)

    perm, qscale = _quad_perm_scale()
    # recurrent weights: K = [o (512); h (512)], h-part halved (h' = 2h)
    W_cat = np.concatenate([W_ih[:, EMB:EMB + HID], 0.5 * np.asarray(W_hh, f32)],
                           axis=1)                              # [2048, 1024]
    wrq = (W_cat.T[:, perm] * qscale[None, :]).astype(bf16)     # [1024, 2048]
    wy_full = np.concatenate([np.asarray(W_ih, f32)[:, :EMB].T,
                              (np.asarray(b_ih, f32) + np.asarray(b_hh, f32))[None, :]],
                             axis=0)                            # [257, 2048ref]
    wyq = (wy_full[:, perm] * qscale[None, :]).astype(bf16)     # [257, 2048]

    whq = (2.0 * np.asarray(W_h, f32).T).astype(bf16)           # [512, 512]
    wcq = (2.0 * np.asarray(W_c, f32).T).astype(bf16)           # [512, 512]
    wgq = np.concatenate([np.asarray(W_g2o, f32).T,
                          np.asarray(b_g2o, f32)[None, :]], 0).astype(bf16)  # [2049,512]
    watq = (0.5 * np.asarray(W_attn, f32).T).astype(bf16)       # [1024, 512]
    wuaq = np.ascontiguousarray(np.asarray(W_u, f32)[:, :LOCAL].T).astype(bf16)
    wuhq = (0.5 * np.asarray(W_u, f32)[:, LOCAL:].T).astype(bf16)  # [512, 512]
    buq = np.asarray(b_u, f32)[None, :].astype(bf16)            # [1, 512]
    wv = np.ascontiguousarray(np.asarray(W_vocab, f32).T).astype(bf16)  # [512,10000]

    # col 2R is an epsilon column (-55 -> exp ~ 1.3e-24): keeps every row's
    # exp-sum nonzero so 1/ssum stays finite on fully-masked (off-diagonal)
    # rows; abd only consumes cols 0:2R so it never reaches vo.
    maskq = np.full((BL, 4, 2 * R + 1), NEG, f32)
    maskq[:, :, 2 * R] = -55.0
    for j in range(4):
        maskq[2 * j, j, 0:R] = 0.0
        maskq[2 * j + 1, j, R:2 * R] = 0.0
    maskq = maskq.astype(bf16)
    idb = np.eye(128, dtype=f32).astype(bf16)
    ones8 = np.ones((1, BL), f32).astype(bf16)

    # teacher-forced input embeddings: y_seq[t] = emb[ans[:, t-1]], y_seq[0]=emb[1]
    idx = np.concatenate([np.full((B, 1), START_IDX, np.int64), ans[:, :-1]], 1)
    y_emb = emb[idx]                                            # [B,T,EMB]

    def chunked(a, rows=128):
        """[K, N] -> [128, K//128, N] (device layout, partition-major),
        zero-padding K up to a multiple of 128."""
        a = np.asarray(a)
        k, n = a.shape
        nch = -(-k // rows)
        if k < nch * rows:
            pad = np.zeros((nch * rows - k, n), a.dtype)
            a = np.concatenate([a, pad], 0)
        return np.ascontiguousarray(
            a.reshape(nch, rows, n).transpose(1, 0, 2))

    shared = {
        "wyq": chunked(wyq), "wrq": chunked(wrq), "whq": chunked(whq),
        "wcq": chunked(wcq), "wgq": chunked(wgq), "watq": chunked(watq),
        "wuaq": chunked(wuaq), "wuhq": chunked(wuhq), "buq": buq,
        "wv": chunked(wv), "maskq": maskq, "idb": idb, "ones8": ones8,
    }
    in_maps = []
    for c in range(NCORES):
        s = slice(c * BL, (c + 1) * BL)
        ye = y_emb[s].transpose(1, 0, 2).reshape(ROWS, EMB)     # [(t,b), EMB]
        yT = np.concatenate([ye.T, np.ones((1, ROWS), f32)], 0).astype(bf16)
        lT = np.ascontiguousarray(L[s].reshape(BL * R, LOCAL).T).astype(bf16)
        qT = np.ascontiguousarray(q[s].T).astype(bf16)          # [512,8]
        gT = np.concatenate([g[s].T, np.ones((1, BL), f32)], 0).astype(bf16)
        d = {"yT": chunked(yT), "lT": chunked(lT), "qT": chunked(qT),
             "gT": chunked(gT)}
        d.update(shared)
        in_maps.append(d)
    return in_maps


def build_nc():
    """Build the per-core Bass graph (identical on all 8 cores)."""
    from concourse import bacc, tile
    import concourse.mybir as mybir

    f32 = mybir.dt.float32
    bf16 = mybir.dt.bfloat16
    AF = mybir.ActivationFunctionType
    ALU = mybir.AluOpType

    nc = bacc.Bacc("TRN2", target_bir_lowering=False, debug=False,
                   num_devices=NCORES)

    def dparam(name, shape, dt=bf16):
        return nc.dram_tensor(name, list(shape), dt, kind="ExternalInput").ap()

    yT_d = dparam("yT", [128, 3, 256])
    wyq_d = dparam("wyq", [128, 3, 2048])
    wrq_d = dparam("wrq", [128, 8, 2048])
    lT_d = dparam("lT", [128, 8, BL * R])
    watq_d = dparam("watq", [128, 8, 512])
    wuaq_d = dparam("wuaq", [128, 8, 512])
    wuhq_d = dparam("wuhq", [128, 4, 512])
    qT_d = dparam("qT", [128, 4, BL])
    whq_d = dparam("whq", [128, 4, 512])
    wcq_d = dparam("wcq", [128, 4, 512])
    gT_d = dparam("gT", [128, 17, BL])
    wgq_d = dparam("wgq", [128, 17, 512])
    buq_d = dparam("buq", [1, 512])
    wv_d = dparam("wv", [128, 4, VOCAB])
    mask_d = dparam("maskq", [BL, 4, 2 * R + 1])
    idb_d = dparam("idb", [128, 128])
    ones8_d = dparam("ones8", [1, BL])
    out_d = nc.dram_tensor("out", [ROWS, VOCAB], bf16, kind="ExternalOutput").ap()

    mm = nc.tensor.matmul
    ADD, MULT = ALU.add, ALU.mult
    stt = nc.vector.scalar_tensor_tensor
    vcp = nc.vector.tensor_copy
    P104 = 104  # 3*32 + 8: spans all four quad groups

    with tile.TileContext(nc) as tc:
        import contextlib
        stack = contextlib.ExitStack()
        with stack:
            pers = stack.enter_context(tc.tile_pool(name="pers", bufs=1))
            stb = stack.enter_context(tc.tile_pool(name="stb", bufs=2))
            gyp = stack.enter_context(tc.tile_pool(name="gyp", bufs=2))
            gyd = stack.enter_context(tc.tile_pool(name="gyd", bufs=1, space="DRAM"))
            pg = stack.enter_context(tc.tile_pool(name="pg", bufs=2, space="PSUM"))
            ptp = stack.enter_context(tc.tile_pool(name="ptp", bufs=2, space="PSUM"))

            dma = nc.sync.dma_start

            # ---- persistent SBUF ----
            wr_sb = pers.tile([128, 8, 2048], bf16)
            wuh_sb = pers.tile([128, 4, 512], bf16)
            apT_sb = pers.tile([128, 4, BL * R], bf16)
            pbd_sb = pers.tile([128, 4, 512], bf16)
            otb_sb = pers.tile([128, 4, T + 1, BL], bf16)
            wv_sb = pers.tile([128, 4, VOCAB], bf16)
            cq_sb = pers.tile([128, 128], f32)
            abd_sb = pers.tile([128, 4, 32], bf16)
            mask_sb = pers.tile([BL, 4, 2 * R + 1], bf16)
            idb_sb = pers.tile([128, 128], bf16)
            gy_dram = gyd.tile([ROWS, 2048], bf16)

            dma(out=mask_sb[:, :, :], in_=mask_d[:, :, :])
            dma(out=idb_sb[:, :], in_=idb_d[:, :])
            dma(out=abd_sb[2 * R:2 * R + 1, 0, 0:BL], in_=ones8_d[:, :])
            dma(out=pbd_sb[2 * R:2 * R + 1, 0, :], in_=buq_d[:, :])
            dma(out=wuh_sb[:, :, :], in_=wuhq_d[:, :, :])
            dma(out=wr_sb[:, :, :], in_=wrq_d[:, :, :])

            id8 = idb_sb[0:8, 0:8]
            idT = idb_sb[0:P104, 0:128]   # zero-padded transpose identity

            def quad_mm(out_t, lhsT, rhs, j, start, stop):
                mm(out_t[32 * j:32 * j + BL, :], lhsT, rhs, start=start,
                   stop=stop, skip_group_check=True, tile_position=(0, 32 * j))

            def transpose104(in_ap, src_pool=None):
                """[104, F] -> ptp tile [128, 4, 32] (cols 32k+b valid)."""
                tp = ptp.tile([128, 4, 32], bf16, name="tp", tag="tp")
                nP = in_ap.shape[0]
                nc.tensor.transpose(tp[0:in_ap.shape[1], :, :], in_ap,
                                    idb_sb[0:nP, 0:128])
                return tp

            # ---- preamble (all DMA issues hoisted before compute) ----
            hT_sb = None
            with tc.tile_pool(name="pre", bufs=1) as pre, \
                 tc.tile_pool(name="pmm", bufs=2, space="PSUM") as pmm:
                q_sb = pre.tile([128, 4, BL], bf16)
                wh_sb = pre.tile([128, 4, 512], bf16)
                wc_sb = pre.tile([128, 4, 512], bf16)
                y_sb = pre.tile([128, 3, 256], bf16)
                wy_sb = pre.tile([128, 3, 2048], bf16)
                lT_sb = pre.tile([128, 8, BL * R], bf16)
                wat_sb = pre.tile([128, 8, 512], bf16)
                wua_sb = pre.tile([128, 8, 512], bf16)
                g_sb = pre.tile([128, 17, BL], bf16)
                wg_sb = pre.tile([128, 17, 512], bf16)
                dma(out=q_sb[:, :, :], in_=qT_d[:, :, :])
                dma(out=wh_sb[:, :, :], in_=whq_d[:, :, :])
                dma(out=wc_sb[:, :, :], in_=wcq_d[:, :, :])
                dma(out=y_sb[:, :, :], in_=yT_d[:, :, :])
                dma(out=wy_sb[:, :, :], in_=wyq_d[:, :, :])
                dma(out=lT_sb[:, :, :], in_=lT_d[:, :, :])
                dma(out=wat_sb[:, :, :], in_=watq_d[:, :, :])
                dma(out=wua_sb[:, :, :], in_=wuaq_d[:, :, :])
                dma(out=g_sb[:, :, :], in_=gT_d[:, :, :])
                dma(out=wg_sb[:, :, :], in_=wgq_d[:, :, :])

                # h0 / c0 (quad)
                hq0 = pmm.tile([128, 128], f32, name="hq0", tag="mmq")
                cq0 = pmm.tile([128, 128], f32, name="cq0", tag="mmq")
                for dst, w_sb in ((hq0, wh_sb), (cq0, wc_sb)):
                    for k in range(4):
                        for j in range(4):
                            quad_mm(dst, q_sb[:, k, :],
                                    w_sb[:, k, 128 * j:128 * (j + 1)], j,
                                    start=(k == 0), stop=(k == 3))
                h_sb = stb.tile([128, 128], bf16, name="h", tag="h")
                vcp(h_sb[0:P104, :], hq0[0:P104, :])
                vcp(cq_sb[0:P104, :], cq0[0:P104, :])
                tp = transpose104(h_sb[0:P104, :])
                hT_sb = stb.tile([128, 4, 32], bf16, name="hT", tag="hT")
                vcp(hT_sb[:, :, 0:BL], tp[:, :, 0:BL])

                # gates_y -> DRAM scratch (bf16, quad-permuted cols)
                for m in range(2):
                    for nb in range(4):
                        ps = pmm.tile([128, 512], f32, name="gyps", tag="mmq")
                        ns = slice(512 * nb, 512 * (nb + 1))
                        for k in range(3):
                            mm(ps[:, :], y_sb[:, k, 128 * m:128 * (m + 1)],
                               wy_sb[:, k, ns], start=(k == 0), stop=(k == 2))
                        lg = stb.tile([128, 512], bf16, name="lg", tag="lg")
                        vcp(lg[:, :], ps[:, :])
                        dma(out=gy_dram[128 * m:128 * (m + 1), ns], in_=lg[:, :])

                # apT + pbd (shared lT)
                for hk in range(4):
                    ps = pmm.tile([128, BL * R], f32, name="apps", tag="mmq")
                    for k in range(8):
                        mm(ps[:, :], wat_sb[:, k, 128 * hk:128 * (hk + 1)],
                           lT_sb[:, k, :], start=(k == 0), stop=(k == 7))
                    vcp(apT_sb[:, hk, :], ps[:, :])
                for mt in range(4):
                    ps = pmm.tile([128, 512], f32, name="pbps", tag="mmq")
                    for k in range(8):
                        mm(ps[0:2 * R, :],
                           lT_sb[:, k, 2 * R * mt:2 * R * (mt + 1)],
                           wua_sb[:, k, :], start=(k == 0), stop=(k == 7))
                    vcp(pbd_sb[0:2 * R, mt, :], ps[0:2 * R, :])

                # o0 (plain [8,512] then 4 narrow transposes into otb slot 0)
                o0ps = pmm.tile([BL, 512], f32, name="o0ps", tag="mmq")
                for k in range(17):
                    mm(o0ps[:, :], g_sb[:, k, :], wg_sb[:, k, :],
                       start=(k == 0), stop=(k == 16))
                o0_sb = stb.tile([BL, 512], bf16, name="o0", tag="o0")
                vcp(o0_sb[:, :], o0ps[:, :])
                tp0 = ptp.tile([128, 4, 32], bf16, name="tp0", tag="tp")
                for k in range(4):
                    nc.tensor.transpose(tp0[:, k, :],
                                        o0_sb[:, 128 * k:128 * (k + 1)],
                                        idb_sb[0:8, 0:32])
                vcp(otb_sb[:, :, 0, :], tp0[:, :, 0:BL])

                # gates(0)
                gytq = gyp.tile([BL, 2048], bf16, name="gytq", tag="gytq")
                dma(out=gytq[:, :], in_=gy_dram[0:BL, :])
                gq = pg.tile([128, 512], f32, name="gq", tag="gq")
                for j in range(4):
                    quad_mm(gq, id8, gytq[0:8, 512 * j:512 * (j + 1)], j,
                            start=True, stop=False)
                for k in range(4):
                    for j in range(4):
                        quad_mm(gq, hT_sb[:, k, 0:BL],
                                wr_sb[:, 4 + k, 512 * j:512 * (j + 1)], j,
                                start=False, stop=False)
                for k in range(4):
                    for j in range(4):
                        quad_mm(gq, otb_sb[:, k, 0, :],
                                wr_sb[:, k, 512 * j:512 * (j + 1)], j,
                                start=False, stop=(k == 3))

            # vocab weight loads: 2 issued per step over the first 16 steps
            wv_pairs = [(vc, k) for vc in range(8) for k in range(4)]

            pe1 = stack.enter_context(tc.tile_pool(name="pe1", bufs=1, space="PSUM"))
            pv = stack.enter_context(tc.tile_pool(name="pv", bufs=1, space="PSUM"))
            pvoc = stack.enter_context(tc.tile_pool(name="pvoc", bufs=2, space="PSUM"))

            # vocab units: (m_tile, col_base, n_size); M-tile 0 interleaved
            nmm = [(0, 512), (512, 512), (1024, 226)]
            units = [(m, 1250 * vc + nb, nsz)
                     for m in range(2) for vc in range(8) for nb, nsz in nmm]

            def vocab_unit(m, cb, nsz):
                ps = pvoc.tile([128, 512], f32, name="vps", tag="vps")
                for k in range(4):
                    mm(ps[:, 0:nsz], otb_sb[:, k, 1 + 16 * m:17 + 16 * m, :],
                       wv_sb[:, k, cb:cb + nsz], start=(k == 0), stop=(k == 3),
                       skip_group_check=True)
                lg = stb.tile([128, 512], bf16, name="lg", tag="lg")
                nc.scalar.copy(lg[:, 0:nsz], ps[:, 0:nsz])
                dma(out=out_d[128 * m:128 * (m + 1), cb:cb + nsz],
                    in_=lg[:, 0:nsz])

            # ---- recurrence ----
            next_unit = 0
            for t in range(T):
                if t < 16:
                    for vc, k in wv_pairs[2 * t:2 * t + 2]:
                        vs = slice(1250 * vc, 1250 * (vc + 1))
                        dma(out=wv_sb[:, k, vs], in_=wv_d[:, k, vs])
                if t < T - 1:
                    gytq = gyp.tile([BL, 2048], bf16, name="gytq", tag="gytq")
                    dma(out=gytq[:, :],
                        in_=gy_dram[BL * (t + 1):BL * (t + 2), :])

                # LSTM cell on quad layout; gq cols per group: [g|i|f|og]*128
                tgall = stb.tile([128, 512], bf16, name="tgall", tag="tgall")
                nc.scalar.activation(tgall[0:P104, 0:256], gq[0:P104, 0:256],
                                     AF.Tanh)
                nc.scalar.activation(tgall[0:P104, 256:512],
                                     gq[0:P104, 256:512], AF.Tanh)
                tg = tgall[0:P104, 0:128]
                ti = tgall[0:P104, 128:256]
                tf_ = tgall[0:P104, 256:384]
                tog = tgall[0:P104, 384:512]
                t1 = stb.tile([128, 128], f32, name="t1", tag="t1")
                stt(t1[0:P104, :], ti, 1.0, tg, op0=ADD, op1=MULT)
                m2 = stb.tile([128, 128], f32, name="m2", tag="m2")
                stt(m2[0:P104, :], tf_, 1.0, cq_sb[0:P104, :], op0=ADD, op1=MULT)
                stt(cq_sb[0:P104, :], m2[0:P104, :], 0.5, t1[0:P104, :],
                    op0=MULT, op1=ADD)
                tc2 = stb.tile([128, 128], bf16, name="tc2", tag="tc2")
                nc.scalar.activation(tc2[0:P104, :], cq_sb[0:P104, :], AF.Tanh,
                                     scale=0.5)
                h_sb = stb.tile([128, 128], bf16, name="h", tag="h")
                stt(h_sb[0:P104, :], tog, 1.0, tc2[0:P104, :], op0=ADD, op1=MULT)

                tp = transpose104(h_sb[0:P104, :])
                hT_sb = stb.tile([128, 4, 32], bf16, name="hT", tag="hT")
                vcp(hT_sb[:, :, 0:BL], tp[:, :, 0:BL])

                # attention E first (critical path), masked via id8 wave
                eq = pe1.tile([128, 2 * R + 1], f32, name="eq", tag="eq")
                for j in range(4):
                    quad_mm(eq, id8, mask_sb[0:8, j, :], j, start=True,
                            stop=False)
                for k in range(4):
                    for j in range(4):
                        mm(eq[32 * j:32 * j + BL, 0:2 * R], hT_sb[:, k, 0:BL],
                           apT_sb[:, k, 2 * R * j:2 * R * (j + 1)],
                           start=False, stop=(k == 3), skip_group_check=True,
                           tile_position=(0, 32 * j))

                # PE filler sized to the exp-chain: gates(t+1) gy + h-part
                if t < T - 1:
                    gq_next = pg.tile([128, 512], f32, name="gq", tag="gq")
                    for j in range(4):
                        quad_mm(gq_next, id8, gytq[0:8, 512 * j:512 * (j + 1)],
                                j, start=True, stop=False)
                    for k in range(4):
                        for j in range(4):
                            quad_mm(gq_next, hT_sb[:, k, 0:BL],
                                    wr_sb[:, 4 + k, 512 * j:512 * (j + 1)], j,
                                    start=False, stop=False)

                expq = stb.tile([128, 2 * R + 1], bf16, name="expq", tag="expq")
                ssum = stb.tile([128, 1], f32, name="ssum", tag="ssum")
                nc.scalar.activation(expq[0:P104, :], eq[0:P104, :], AF.Exp,
                                     accum_out=ssum[0:P104, :])
                rs = stb.tile([128, 1], f32, name="rs", tag="rs")
                nc.vector.reciprocal(rs[0:P104, :], ssum[0:P104, :])
                attq = stb.tile([128, 2 * R], bf16, name="attq", tag="attq")
                nc.vector.tensor_scalar_mul(attq[0:P104, :],
                                            expq[0:P104, 0:2 * R],
                                            rs[0:P104, :])
                tpE = transpose104(attq[0:P104, :])
                vcp(abd_sb[0:2 * R, :, 0:BL], tpE[0:2 * R, :, 0:BL])

                # vo: h-part runs right after attT (hides the abd copy),
                # then b_u + att-weighted P
                vq = pv.tile([128, 128], f32, name="vq", tag="vq")
                for k in range(4):
                    for j in range(4):
                        quad_mm(vq, hT_sb[:, k, 0:BL],
                                wuh_sb[:, k, 128 * j:128 * (j + 1)], j,
                                start=(k == 0), stop=False)
                for m in range(4):
                    kk = 2 * R + 1 if m == 0 else 2 * R
                    for j in range(4):
                        quad_mm(vq, abd_sb[0:kk, m, 0:BL],
                                pbd_sb[0:kk, m, 128 * j:128 * (j + 1)], j,
                                start=False, stop=(m == 3))
                o_sb = stb.tile([128, 128], bf16, name="o", tag="o")
                nc.scalar.activation(o_sb[0:P104, :], vq[0:P104, :], AF.Tanh)

                tpo = transpose104(o_sb[0:P104, :])
                vcp(otb_sb[:, :, t + 1, :], tpo[:, :, 0:BL])

                # gates(t+1): o-part
                if t < T - 1:
                    for k in range(4):
                        for j in range(4):
                            quad_mm(gq_next, otb_sb[:, k, t + 1, :],
                                    wr_sb[:, k, 512 * j:512 * (j + 1)], j,
                                    start=False, stop=(k == 3))
                    gq = gq_next

                # M-tile-0 vocab units fill the next cell's PE-idle window
                if t >= 16:
                    n_units = 2 if t >= 24 else 1
                    for _ in range(n_units):
                        vocab_unit(*units[next_unit])
                        next_unit += 1

            # ---- vocab M-tile 1 ----
            for u in range(next_unit, 48):
                vocab_unit(*units[u])

    nc.compile()
    return nc


_STATE = {}


def kernel(**inputs):
    from concourse.bass_utils import run_bass_kernel_spmd

    in_maps = prep_inputs(**inputs)
    if "nc" not in _STATE:
        _STATE["nc"] = build_nc()
    nc = _STATE["nc"]
    res = run_bass_kernel_spmd(nc, in_maps, core_ids=list(range(NCORES)))
    bv = np.asarray(inputs["b_vocab"], np.float32)
    full = np.empty((B, T, VOCAB), np.float32)
    for c in range(NCORES):
        o = res.results[c]["out"].astype(np.float32) + bv[None, :]
        full[c * BL:(c + 1) * BL] = o.reshape(T, BL, VOCAB).transpose(1, 0, 2)
    return full


# revision 30
# speedup vs baseline: 1.2738x; 1.0136x over previous
"""AnswerDecoder (LSTM decoder w/ visual attention) on 8 TRN2 NeuronCores.

Strategy: pure data-parallel over batch (8 batches/core), zero collectives.

v2 design notes (all relative to the measured v1 trace: 787us, PE-bound):
- "Quad" layout: partition 32j+b holds (hidden-slice j, batch b). All LSTM
  cell elementwise work runs as single [104, N] instructions instead of 4-8
  [8, N] ones (ACT/DVE cost is free-dim-dominated).
- 4-way PE column tiling: the four quad groups' matmuls use tile_position
  (0, 32j) and stream concurrently (measured 82.5 ns/MM for N=512 bf16 vs
  231.8 serial). Same-bank disjoint-partition accumulation verified OK.
- bf16 weights/activations everywhere on the PE (f32r at N<256 runs at 1/4
  rate; bf16 is 1 cycle/row always). c-state and PSUM stay f32.
- One batched PE transpose per h/att/o per step ([104,128] -> [128,128] via
  zero-padded identity) instead of 4 narrow transposes each.
- Softmax without max-subtraction (|e| << 80 so fp32 exp is safe); the
  block-diagonal mask is folded into the e-matmul as a K=8 identity wave;
  row sums come free from the Exp accumulator; normalization happens on the
  exp output before transposing.
- W_u bias enters via a ones-row in abd x b_u row in pbd; vocab bias is
  added on the host.
- Vocab projection: M-tile 0 (steps 0-15) is interleaved into PE idle slots
  of steps 16-31; only M-tile 1 runs after the loop. Output is bf16
  (upcast on host).
"""

import numpy as np

B, T, R = 64, 32, 49
LOCAL, QVEC, EMB, HID, VOCAB = 1024, 512, 256, 512, 10000
START_IDX = 1
NCORES = 8
BL = B // NCORES        # 8 batches per core
ROWS = T * BL           # 256 output rows per core, t-major (row = t*BL + b)
NEG = -60000.0          # mask value; exp(NEG + e) underflows to 0 in fp32


def _quad_perm_scale():
    """Device gate-column order: group j, gate [g,i,f,og], offset f.
    Returns (perm, scale): device col -> ref 4H row, and the 0.5 tanh-half
    scaling for i/f/og. Order [g,i,...] lets the cell tanh split into two
    contiguous 256-col ops with (g,i) first so t1 starts early."""
    # ref row ranges: i 0:512, f 512:1024, g 1024:1536, o 1536:2048
    base = {0: 1024, 1: 0, 2: 512, 3: 1536}      # device gate idx -> ref base
    perm = np.empty(2048, np.int64)
    scale = np.empty(2048, np.float32)
    for j in range(4):
        for g in range(4):
            cols = slice(j * 512 + g * 128, j * 512 + g * 128 + 128)
            perm[cols] = base[g] + 128 * j + np.arange(128)
            scale[cols] = 1.0 if g == 0 else 0.5
    return perm, scale


def prep_inputs(image_local_features, image_global_features, question_vectors,
                answers, emb, W_g2o, b_g2o, W_h, W_c, W_ih, W_hh, b_ih, b_hh,
                W_attn, W_u, b_u, W_vocab, b_vocab):
    """Host-side data layout prep. Returns list of per-core input dicts."""
    import ml_dtypes
    bf16 = ml_dtypes.bfloat16
    f32 = np.float32

    L = np.asarray(image_local_features, f32)                   # [B,R,F]
    g = np.asarray(image_global_features, f32)                  # [B,2F]
    q = np.asarray(question_vectors, f32)                       # [B,Q]
    ans = np.asarray(answers).astype(np.int64)                  # [B,T]
    emb = np.asarray(emb, f32)

    perm, qscale = _quad_perm_scale()
    # recurrent weights: K = [o (512); h (512)], h-part halved (h' = 2h)
    W_cat = np.concatenate([W_ih[:, EMB:EMB + HID], 0.5 * np.asarray(W_hh, f32)],
                           axis=1)                              # [2048, 1024]
    wrq = (W_cat.T[:, perm] * qscale[None, :]).astype(bf16)     # [1024, 2048]
    wy_full = np.concatenate([np.asarray(W_ih, f32)[:, :EMB].T,
                              (np.asarray(b_ih, f32) + np.asarray(b_hh, f32))[None, :]],
                             axis=0)                            # [257, 2048ref]
    wyq = (wy_full[:, perm] * qscale[None, :]).astype(bf16)     # [257, 2048]

    whq = (2.0 * np.asarray(W_h, f32).T).astype(bf16)           # [512, 512]
    wcq = (2.0 * np.asarray(W_c, f32).T).astype(bf16)           # [512, 512]
    wgq = np.concatenate([np.asarray(W_g2o, f32).T,
                          np.asarray(b_g2o, f32)[None, :]], 0).astype(bf16)  # [2049,512]
    watq = (0.5 * np.asarray(W_attn, f32).T).astype(bf16)       # [1024, 512]
    wuaq = np.ascontiguousarray(np.asarray(W_u, f32)[:, :LOCAL].T).astype(bf16)
    wuhq = (0.5 * np.asarray(W_u, f32)[:, LOCAL:].T).astype(bf16)  # [512, 512]
    buq = np.asarray(b_u, f32)[None, :].astype(bf16)            # [1, 512]
    wv = np.ascontiguousarray(np.asarray(W_vocab, f32).T).astype(bf16)  # [512,10000]

    # col 2R is an epsilon column (-55 -> exp ~ 1.3e-24): keeps every row's
    # exp-sum nonzero so 1/ssum stays finite on fully-masked (off-diagonal)
    # rows; abd only consumes cols 0:2R so it never reaches vo.
    maskq = np.full((BL, 4, 2 * R + 1), NEG, f32)
    maskq[:, :, 2 * R] = -55.0
    for j in range(4):
        maskq[2 * j, j, 0:R] = 0.0
        maskq[2 * j + 1, j, R:2 * R] = 0.0
    maskq = maskq.astype(bf16)
    idb = np.eye(128, dtype=f32).astype(bf16)
    ones8 = np.ones((1, BL), f32).astype(bf16)

    # teacher-forced input embeddings: y_seq[t] = emb[ans[:, t-1]], y_seq[0]=emb[1]
    idx = np.concatenate([np.full((B, 1), START_IDX, np.int64), ans[:, :-1]], 1)
    y_emb = emb[idx]                                            # [B,T,EMB]

    def chunked(a, rows=128):
        """[K, N] -> [128, K//128, N] (device layout, partition-major),
        zero-padding K up to a multiple of 128."""
        a = np.asarray(a)
        k, n = a.shape
        nch = -(-k // rows)
        if k < nch * rows:
            pad = np.zeros((nch * rows - k, n), a.dtype)
            a = np.concatenate([a, pad], 0)
        return np.ascontiguousarray(
            a.reshape(nch, rows, n).transpose(1, 0, 2))

    shared = {
        "wyq": chunked(wyq), "wrq": chunked(wrq), "whq": chunked(whq),
        "wcq": chunked(wcq), "wgq": chunked(wgq), "watq": chunked(watq),
        "wuaq": chunked(wuaq), "wuhq": chunked(wuhq), "buq": buq,
        "wv": chunked(wv), "maskq": maskq, "idb": idb, "ones8": ones8,
    }
    in_maps = []
    for c in range(NCORES):
        s = slice(c * BL, (c + 1) * BL)
        ye = y_emb[s].transpose(1, 0, 2).reshape(ROWS, EMB)     # [(t,b), EMB]
        yT = np.concatenate([ye.T, np.ones((1, ROWS), f32)], 0).astype(bf16)
        lT = np.ascontiguousarray(L[s].reshape(BL * R, LOCAL).T).astype(bf16)
        qT = np.ascontiguousarray(q[s].T).astype(bf16)          # [512,8]
        gT = np.concatenate([g[s].T, np.ones((1, BL), f32)], 0).astype(bf16)
        d = {"yT": chunked(yT), "lT": chunked(lT), "qT": chunked(qT),
             "gT": chunked(gT)}
        d.update(shared)
        in_maps.append(d)
    return in_maps


def build_nc():
    """Build the per-core Bass graph (identical on all 8 cores)."""
    from concourse import bacc, tile
    import concourse.mybir as mybir

    f32 = mybir.dt.float32
    bf16 = mybir.dt.bfloat16
    AF = mybir.ActivationFunctionType
    ALU = mybir.AluOpType

    nc = bacc.Bacc("TRN2", target_bir_lowering=False, debug=False,
                   num_devices=NCORES)

    def dparam(name, shape, dt=bf16):
        return nc.dram_tensor(name, list(shape), dt, kind="ExternalInput").ap()

    yT_d = dparam("yT", [128, 3, 256])
    wyq_d = dparam("wyq", [128, 3, 2048])
    wrq_d = dparam("wrq", [128, 8, 2048])
    lT_d = dparam("lT", [128, 8, BL * R])
    watq_d = dparam("watq", [128, 8, 512])
    wuaq_d = dparam("wuaq", [128, 8, 512])
    wuhq_d = dparam("wuhq", [128, 4, 512])
    qT_d = dparam("qT", [128, 4, BL])
    whq_d = dparam("whq", [128, 4, 512])
    wcq_d = dparam("wcq", [128, 4, 512])
    gT_d = dparam("gT", [128, 17, BL])
    wgq_d = dparam("wgq", [128, 17, 512])
    buq_d = dparam("buq", [1, 512])
    wv_d = dparam("wv", [128, 4, VOCAB])
    mask_d = dparam("maskq", [BL, 4, 2 * R + 1])
    idb_d = dparam("idb", [128, 128])
    ones8_d = dparam("ones8", [1, BL])
    out_d = nc.dram_tensor("out", [ROWS, VOCAB], bf16, kind="ExternalOutput").ap()

    mm = nc.tensor.matmul
    ADD, MULT = ALU.add, ALU.mult
    stt = nc.vector.scalar_tensor_tensor
    vcp = nc.vector.tensor_copy
    P104 = 104  # 3*32 + 8: spans all four quad groups

    with tile.TileContext(nc) as tc:
        import contextlib
        stack = contextlib.ExitStack()
        with stack:
            pers = stack.enter_context(tc.tile_pool(name="pers", bufs=1))
            stb = stack.enter_context(tc.tile_pool(name="stb", bufs=2))
            gyp = stack.enter_context(tc.tile_pool(name="gyp", bufs=2))
            gyd = stack.enter_context(tc.tile_pool(name="gyd", bufs=1, space="DRAM"))
            pg = stack.enter_context(tc.tile_pool(name="pg", bufs=2, space="PSUM"))
            ptp = stack.enter_context(tc.tile_pool(name="ptp", bufs=2, space="PSUM"))

            dma = nc.sync.dma_start

            # ---- persistent SBUF ----
            wr_sb = pers.tile([128, 8, 2048], bf16)
            wuh_sb = pers.tile([128, 4, 512], bf16)
            apT_sb = pers.tile([128, 4, BL * R], bf16)
            pbd_sb = pers.tile([128, 4, 512], bf16)
            otb_sb = pers.tile([128, 4, T + 1, BL], bf16)
            wv_sb = pers.tile([128, 4, VOCAB], bf16)
            cq_sb = pers.tile([128, 128], f32)
            abd_sb = pers.tile([128, 4, 32], bf16)
            mask_sb = pers.tile([BL, 4, 2 * R + 1], bf16)
            idb_sb = pers.tile([128, 128], bf16)
            gy_dram = gyd.tile([ROWS, 2048], bf16)

            dma(out=mask_sb[:, :, :], in_=mask_d[:, :, :])
            dma(out=idb_sb[:, :], in_=idb_d[:, :])
            dma(out=abd_sb[2 * R:2 * R + 1, 0, 0:BL], in_=ones8_d[:, :])
            dma(out=pbd_sb[2 * R:2 * R + 1, 0, :], in_=buq_d[:, :])
            dma(out=wuh_sb[:, :, :], in_=wuhq_d[:, :, :])
            dma(out=wr_sb[:, :, :], in_=wrq_d[:, :, :])

            id8 = idb_sb[0:8, 0:8]
            idT = idb_sb[0:P104, 0:128]   # zero-padded transpose identity

            def quad_mm(out_t, lhsT, rhs, j, start, stop):
                mm(out_t[32 * j:32 * j + BL, :], lhsT, rhs, start=start,
                   stop=stop, skip_group_check=True, tile_position=(0, 32 * j))

            def transpose104(in_ap, src_pool=None):
                """[104, F] -> ptp tile [128, 4, 32] (cols 32k+b valid)."""
                tp = ptp.tile([128, 4, 32], bf16, name="tp", tag="tp")
                nP = in_ap.shape[0]
                nc.tensor.transpose(tp[0:in_ap.shape[1], :, :], in_ap,
                                    idb_sb[0:nP, 0:128])
                return tp

            # ---- preamble (all DMA issues hoisted before compute) ----
            hT_sb = None
            with tc.tile_pool(name="pre", bufs=1) as pre, \
                 tc.tile_pool(name="pmm", bufs=2, space="PSUM") as pmm:
                q_sb = pre.tile([128, 4, BL], bf16)
                wh_sb = pre.tile([128, 4, 512], bf16)
                wc_sb = pre.tile([128, 4, 512], bf16)
                y_sb = pre.tile([128, 3, 256], bf16)
                wy_sb = pre.tile([128, 3, 2048], bf16)
                lT_sb = pre.tile([128, 8, BL * R], bf16)
                wat_sb = pre.tile([128, 8, 512], bf16)
                wua_sb = pre.tile([128, 8, 512], bf16)
                g_sb = pre.tile([128, 17, BL], bf16)
                wg_sb = pre.tile([128, 17, 512], bf16)
                dma(out=q_sb[:, :, :], in_=qT_d[:, :, :])
                dma(out=wh_sb[:, :, :], in_=whq_d[:, :, :])
                dma(out=wc_sb[:, :, :], in_=wcq_d[:, :, :])
                dma(out=y_sb[:, :, :], in_=yT_d[:, :, :])
                dma(out=wy_sb[:, :, :], in_=wyq_d[:, :, :])
                dma(out=lT_sb[:, :, :], in_=lT_d[:, :, :])
                dma(out=wat_sb[:, :, :], in_=watq_d[:, :, :])
                dma(out=wua_sb[:, :, :], in_=wuaq_d[:, :, :])
                dma(out=g_sb[:, :, :], in_=gT_d[:, :, :])
                dma(out=wg_sb[:, :, :], in_=wgq_d[:, :, :])

                # h0 / c0 (quad)
                hq0 = pmm.tile([128, 128], f32, name="hq0", tag="mmq")
                cq0 = pmm.tile([128, 128], f32, name="cq0", tag="mmq")
                for dst, w_sb in ((hq0, wh_sb), (cq0, wc_sb)):
                    for k in range(4):
                        for j in range(4):
                            quad_mm(dst, q_sb[:, k, :],
                                    w_sb[:, k, 128 * j:128 * (j + 1)], j,
                                    start=(k == 0), stop=(k == 3))
                h_sb = stb.tile([128, 128], bf16, name="h", tag="h")
                vcp(h_sb[0:P104, :], hq0[0:P104, :])
                vcp(cq_sb[0:P104, :], cq0[0:P104, :])
                tp = transpose104(h_sb[0:P104, :])
                hT_sb = stb.tile([128, 4, 32], bf16, name="hT", tag="hT")
                vcp(hT_sb[:, :, 0:BL], tp[:, :, 0:BL])

                # gates_y -> DRAM scratch (bf16, quad-permuted cols)
                for m in range(2):
                    for nb in range(4):
                        ps = pmm.tile([128, 512], f32, name="gyps", tag="mmq")
                        ns = slice(512 * nb, 512 * (nb + 1))
                        for k in range(3):
                            mm(ps[:, :], y_sb[:, k, 128 * m:128 * (m + 1)],
                               wy_sb[:, k, ns], start=(k == 0), stop=(k == 2))
                        lg = stb.tile([128, 512], bf16, name="lg", tag="lg")
                        vcp(lg[:, :], ps[:, :])
                        dma(out=gy_dram[128 * m:128 * (m + 1), ns], in_=lg[:, :])

                # apT + pbd (shared lT)
                for hk in range(4):
                    ps = pmm.tile([128, BL * R], f32, name="apps", tag="mmq")
                    for k in range(8):
                        mm(ps[:, :], wat_sb[:, k, 128 * hk:128 * (hk + 1)],
                           lT_sb[:, k, :], start=(k == 0), stop=(k == 7))
                    vcp(apT_sb[:, hk, :], ps[:, :])
                for mt in range(4):
                    ps = pmm.tile([128, 512], f32, name="pbps", tag="mmq")
                    for k in range(8):
                        mm(ps[0:2 * R, :],
                           lT_sb[:, k, 2 * R * mt:2 * R * (mt + 1)],
                           wua_sb[:, k, :], start=(k == 0), stop=(k == 7))
                    vcp(pbd_sb[0:2 * R, mt, :], ps[0:2 * R, :])

                # o0 (plain [8,512] then 4 narrow transposes into otb slot 0)
                o0ps = pmm.tile([BL, 512], f32, name="o0ps", tag="mmq")
                for k in range(17):
                    mm(o0ps[:, :], g_sb[:, k, :], wg_sb[:, k, :],
                       start=(k == 0), stop=(k == 16))
                o0_sb = stb.tile([BL, 512], bf16, name="o0", tag="o0")
                vcp(o0_sb[:, :], o0ps[:, :])
                tp0 = ptp.tile([128, 4, 32], bf16, name="tp0", tag="tp")
                for k in range(4):
                    nc.tensor.transpose(tp0[:, k, :],
                                        o0_sb[:, 128 * k:128 * (k + 1)],
                                        idb_sb[0:8, 0:32])
                vcp(otb_sb[:, :, 0, :], tp0[:, :, 0:BL])

                # gates(0)
                gytq = gyp.tile([BL, 2048], bf16, name="gytq", tag="gytq")
                dma(out=gytq[:, :], in_=gy_dram[0:BL, :])
                gq = pg.tile([128, 512], f32, name="gq", tag="gq")
                for j in range(4):
                    quad_mm(gq, id8, gytq[0:8, 512 * j:512 * (j + 1)], j,
                            start=True, stop=False)
                for k in range(4):
                    for j in range(4):
                        quad_mm(gq, hT_sb[:, k, 0:BL],
                                wr_sb[:, 4 + k, 512 * j:512 * (j + 1)], j,
                                start=False, stop=False)
                for k in range(4):
                    for j in range(4):
                        quad_mm(gq, otb_sb[:, k, 0, :],
                                wr_sb[:, k, 512 * j:512 * (j + 1)], j,
                                start=False, stop=(k == 3))

            # vocab weight loads: 2 issued per step over the first 16 steps
            wv_pairs = [(vc, k) for vc in range(8) for k in range(4)]

            pe1 = stack.enter_context(tc.tile_pool(name="pe1", bufs=1, space="PSUM"))
            pv = stack.enter_context(tc.tile_pool(name="pv", bufs=1, space="PSUM"))
            pvoc = stack.enter_context(tc.tile_pool(name="pvoc", bufs=2, space="PSUM"))

            # vocab units: (m_tile, col_base, n_size); M-tile 0 interleaved
            nmm = [(0, 512), (512, 512), (1024, 226)]
            units = [(m, 1250 * vc + nb, nsz)
                     for m in range(2) for vc in range(8) for nb, nsz in nmm]

            def vocab_unit(m, cb, nsz, eng=0):
                ps = pvoc.tile([128, 512], f32, name="vps", tag="vps")
                for k in range(4):
                    mm(ps[:, 0:nsz], otb_sb[:, k, 1 + 16 * m:17 + 16 * m, :],
                       wv_sb[:, k, cb:cb + nsz], start=(k == 0), stop=(k == 3),
                       skip_group_check=True)
                lg = stb.tile([128, 512], bf16, name="lg", tag="lg")
                if eng:
                    vcp(lg[:, 0:nsz], ps[:, 0:nsz])
                else:
                    nc.scalar.copy(lg[:, 0:nsz], ps[:, 0:nsz])
                dma(out=out_d[128 * m:128 * (m + 1), cb:cb + nsz],
                    in_=lg[:, 0:nsz])

            # PE keep-warm filler: HAM re-throttles to 1.2 GHz whenever PE
            # duty drops across its 3.4us window; these dead matmuls (into a
            # scratch PSUM, consumed once post-loop) hold K=8/8 through the
            # ACT/DVE-bound cell phase.
            scr_dram = gyd.tile([BL, 512], bf16, name="scr_dram")

            def keep_warm(n):
                ps = pvoc.tile([128, 512], f32, name="vps", tag="vps")
                for k in range(n):
                    mm(ps[0:BL, :], idb_sb[:, 0:BL], wr_sb[:, k, 0:512],
                       start=(k == 0), stop=(k == n - 1),
                       skip_group_check=True)
                return ps

            warm_ps = []

            # ---- recurrence ----
            next_unit = 0
            for t in range(T):
                if t < 16:
                    for vc, k in wv_pairs[2 * t:2 * t + 2]:
                        vs = slice(1250 * vc, 1250 * (vc + 1))
                        dma(out=wv_sb[:, k, vs], in_=wv_d[:, k, vs])
                if t < T - 1:
                    gytq = gyp.tile([BL, 2048], bf16, name="gytq", tag="gytq")
                    dma(out=gytq[:, :],
                        in_=gy_dram[BL * (t + 1):BL * (t + 2), :])

                # fill the ACT/DVE-bound cell window with PE work
                keep_warm(8 if t < 16 else 3)

                # LSTM cell on quad layout; gq cols per group: [g|i|f|og]*128
                tgall = stb.tile([128, 512], bf16, name="tgall", tag="tgall")
                nc.scalar.activation(tgall[0:P104, 0:256], gq[0:P104, 0:256],
                                     AF.Tanh)
                nc.scalar.activation(tgall[0:P104, 256:512],
                                     gq[0:P104, 256:512], AF.Tanh)
                tg = tgall[0:P104, 0:128]
                ti = tgall[0:P104, 128:256]
                tf_ = tgall[0:P104, 256:384]
                tog = tgall[0:P104, 384:512]
                t1 = stb.tile([128, 128], f32, name="t1", tag="t1")
                stt(t1[0:P104, :], ti, 1.0, tg, op0=ADD, op1=MULT)
                m2 = stb.tile([128, 128], f32, name="m2", tag="m2")
                stt(m2[0:P104, :], tf_, 1.0, cq_sb[0:P104, :], op0=ADD, op1=MULT)
                stt(cq_sb[0:P104, :], m2[0:P104, :], 0.5, t1[0:P104, :],
                    op0=MULT, op1=ADD)
                tc2 = stb.tile([128, 128], bf16, name="tc2", tag="tc2")
                nc.scalar.activation(tc2[0:P104, :], cq_sb[0:P104, :], AF.Tanh,
                                     scale=0.5)
                h_sb = stb.tile([128, 128], bf16, name="h", tag="h")
                stt(h_sb[0:P104, :], tog, 1.0, tc2[0:P104, :], op0=ADD, op1=MULT)

                tp = transpose104(h_sb[0:P104, :])
                hT_sb = stb.tile([128, 4, 32], bf16, name="hT", tag="hT")
                vcp(hT_sb[:, :, 0:BL], tp[:, :, 0:BL])

                # attention E first (critical path), masked via id8 wave
                eq = pe1.tile([128, 2 * R + 1], f32, name="eq", tag="eq")
                for j in range(4):
                    quad_mm(eq, id8, mask_sb[0:8, j, :], j, start=True,
                            stop=False)
                for k in range(4):
                    for j in range(4):
                        mm(eq[32 * j:32 * j + BL, 0:2 * R], hT_sb[:, k, 0:BL],
                           apT_sb[:, k, 2 * R * j:2 * R * (j + 1)],
                           start=False, stop=(k == 3), skip_group_check=True,
                           tile_position=(0, 32 * j))

                # PE filler sized to the exp-chain: gates(t+1) gy + h-part
                if t < T - 1:
                    gq_next = pg.tile([128, 512], f32, name="gq", tag="gq")
                    for j in range(4):
                        quad_mm(gq_next, id8, gytq[0:8, 512 * j:512 * (j + 1)],
                                j, start=True, stop=False)
                    for k in range(4):
                        for j in range(4):
                            quad_mm(gq_next, hT_sb[:, k, 0:BL],
                                    wr_sb[:, 4 + k, 512 * j:512 * (j + 1)], j,
                                    start=False, stop=False)
                keep_warm(2)

                expq = stb.tile([128, 2 * R + 1], bf16, name="expq", tag="expq")
                ssum = stb.tile([128, 1], f32, name="ssum", tag="ssum")
                nc.scalar.activation(expq[0:P104, :], eq[0:P104, :], AF.Exp,
                                     accum_out=ssum[0:P104, :])
                rs = stb.tile([128, 1], f32, name="rs", tag="rs")
                nc.vector.reciprocal(rs[0:P104, :], ssum[0:P104, :])
                attq = stb.tile([128, 2 * R], bf16, name="attq", tag="attq")
                nc.vector.tensor_scalar_mul(attq[0:P104, :],
                                            expq[0:P104, 0:2 * R],
                                            rs[0:P104, :])
                tpE = transpose104(attq[0:P104, :])
                vcp(abd_sb[0:2 * R, :, 0:BL], tpE[0:2 * R, :, 0:BL])

                # vo: h-part runs right after attT (hides the abd copy),
                # then b_u + att-weighted P
                vq = pv.tile([128, 128], f32, name="vq", tag="vq")
                for k in range(4):
                    for j in range(4):
                        quad_mm(vq, hT_sb[:, k, 0:BL],
                                wuh_sb[:, k, 128 * j:128 * (j + 1)], j,
                                start=(k == 0), stop=False)
                for m in range(4):
                    kk = 2 * R + 1 if m == 0 else 2 * R
                    for j in range(4):
                        quad_mm(vq, abd_sb[0:kk, m, 0:BL],
                                pbd_sb[0:kk, m, 128 * j:128 * (j + 1)], j,
                                start=False, stop=(m == 3))
                o_sb = stb.tile([128, 128], bf16, name="o", tag="o")
                nc.scalar.activation(o_sb[0:P104, :], vq[0:P104, :], AF.Tanh)

                tpo = transpose104(o_sb[0:P104, :])
                vcp(otb_sb[:, :, t + 1, :], tpo[:, :, 0:BL])

                # gates(t+1): o-part
                if t < T - 1:
                    for k in range(4):
                        for j in range(4):
                            quad_mm(gq_next, otb_sb[:, k, t + 1, :],
                                    wr_sb[:, k, 512 * j:512 * (j + 1)], j,
                                    start=False, stop=(k == 3))
                    gq = gq_next

                # M-tile-0 vocab units fill the next cell's PE-idle window
                if t >= 16:
                    n_units = 2 if t >= 24 else 1
                    for _ in range(n_units):
                        vocab_unit(*units[next_unit], eng=next_unit % 2)
                        next_unit += 1

            # consume the last keep-warm scratch so the chain stays live
            wlast = keep_warm(1)
            wsb = stb.tile([BL, 512], bf16, name="wsb", tag="o0")
            vcp(wsb[:, :], wlast[0:BL, :])
            dma(out=scr_dram[:, :], in_=wsb[:, :])

            # ---- vocab M-tile 1 ----
            for u in range(next_unit, 48):
                vocab_unit(*units[u], eng=u % 2)

    nc.compile()
    return nc


_STATE = {}


def kernel(**inputs):
    from concourse.bass_utils import run_bass_kernel_spmd

    in_maps = prep_inputs(**inputs)
    if "nc" not in _STATE:
        _STATE["nc"] = build_nc()
    nc = _STATE["nc"]
    res = run_bass_kernel_spmd(nc, in_maps, core_ids=list(range(NCORES)))
    bv = np.asarray(inputs["b_vocab"], np.float32)
    full = np.empty((B, T, VOCAB), np.float32)
    for c in range(NCORES):
        o = res.results[c]["out"].astype(np.float32) + bv[None, :]
        full[c * BL:(c + 1) * BL] = o.reshape(T, BL, VOCAB).transpose(1, 0, 2)
    return full


# revision 34
# speedup vs baseline: 1.3000x; 1.0205x over previous
"""AnswerDecoder (LSTM decoder w/ visual attention) on 8 TRN2 NeuronCores.

Strategy: pure data-parallel over batch (8 batches/core), zero collectives.

v2 design notes (all relative to the measured v1 trace: 787us, PE-bound):
- "Quad" layout: partition 32j+b holds (hidden-slice j, batch b). All LSTM
  cell elementwise work runs as single [104, N] instructions instead of 4-8
  [8, N] ones (ACT/DVE cost is free-dim-dominated).
- 4-way PE column tiling: the four quad groups' matmuls use tile_position
  (0, 32j) and stream concurrently (measured 82.5 ns/MM for N=512 bf16 vs
  231.8 serial). Same-bank disjoint-partition accumulation verified OK.
- bf16 weights/activations everywhere on the PE (f32r at N<256 runs at 1/4
  rate; bf16 is 1 cycle/row always). c-state and PSUM stay f32.
- One batched PE transpose per h/att/o per step ([104,128] -> [128,128] via
  zero-padded identity) instead of 4 narrow transposes each.
- Softmax without max-subtraction (|e| << 80 so fp32 exp is safe); the
  block-diagonal mask is folded into the e-matmul as a K=8 identity wave;
  row sums come free from the Exp accumulator; normalization happens on the
  exp output before transposing.
- W_u bias enters via a ones-row in abd x b_u row in pbd; vocab bias is
  added on the host.
- Vocab projection: M-tile 0 (steps 0-15) is interleaved into PE idle slots
  of steps 16-31; only M-tile 1 runs after the loop. Output is bf16
  (upcast on host).
"""

import numpy as np

B, T, R = 64, 32, 49
LOCAL, QVEC, EMB, HID, VOCAB = 1024, 512, 256, 512, 10000
START_IDX = 1
NCORES = 8
BL = B // NCORES        # 8 batches per core
ROWS = T * BL           # 256 output rows per core, t-major (row = t*BL + b)
NEG = -60000.0          # mask value; exp(NEG + e) underflows to 0 in fp32


def _quad_perm_scale():
    """Device gate-column order: group j, gate [g,i,f,og], offset f.
    Returns (perm, scale): device col -> ref 4H row, and the 0.5 tanh-half
    scaling for i/f/og. Order [g,i,...] lets the cell tanh split into two
    contiguous 256-col ops with (g,i) first so t1 starts early."""
    # ref row ranges: i 0:512, f 512:1024, g 1024:1536, o 1536:2048
    base = {0: 1024, 1: 0, 2: 512, 3: 1536}      # device gate idx -> ref base
    perm = np.empty(2048, np.int64)
    scale = np.empty(2048, np.float32)
    for j in range(4):
        for g in range(4):
            cols = slice(j * 512 + g * 128, j * 512 + g * 128 + 128)
            perm[cols] = base[g] + 128 * j + np.arange(128)
            scale[cols] = 1.0 if g == 0 else 0.5
    return perm, scale


def prep_inputs(image_local_features, image_global_features, question_vectors,
                answers, emb, W_g2o, b_g2o, W_h, W_c, W_ih, W_hh, b_ih, b_hh,
                W_attn, W_u, b_u, W_vocab, b_vocab):
    """Host-side data layout prep. Returns list of per-core input dicts."""
    import ml_dtypes
    bf16 = ml_dtypes.bfloat16
    f32 = np.float32

    L = np.asarray(image_local_features, f32)                   # [B,R,F]
    g = np.asarray(image_global_features, f32)                  # [B,2F]
    q = np.asarray(question_vectors, f32)                       # [B,Q]
    ans = np.asarray(answers).astype(np.int64)                  # [B,T]
    emb = np.asarray(emb, f32)

    perm, qscale = _quad_perm_scale()
    # recurrent weights: K = [o (512); h (512)], h-part halved (h' = 2h)
    W_cat = np.concatenate([W_ih[:, EMB:EMB + HID], 0.5 * np.asarray(W_hh, f32)],
                           axis=1)                              # [2048, 1024]
    wrq = (W_cat.T[:, perm] * qscale[None, :]).astype(bf16)     # [1024, 2048]
    wy_full = np.concatenate([np.asarray(W_ih, f32)[:, :EMB].T,
                              (np.asarray(b_ih, f32) + np.asarray(b_hh, f32))[None, :]],
                             axis=0)                            # [257, 2048ref]
    wyq = (wy_full[:, perm] * qscale[None, :]).astype(bf16)     # [257, 2048]

    whq = (2.0 * np.asarray(W_h, f32).T).astype(bf16)           # [512, 512]
    wcq = (2.0 * np.asarray(W_c, f32).T).astype(bf16)           # [512, 512]
    wgq = np.concatenate([np.asarray(W_g2o, f32).T,
                          np.asarray(b_g2o, f32)[None, :]], 0).astype(bf16)  # [2049,512]
    watq = (0.5 * np.asarray(W_attn, f32).T).astype(bf16)       # [1024, 512]
    wuaq = np.ascontiguousarray(np.asarray(W_u, f32)[:, :LOCAL].T).astype(bf16)
    wuhq = (0.5 * np.asarray(W_u, f32)[:, LOCAL:].T).astype(bf16)  # [512, 512]
    buq = np.asarray(b_u, f32)[None, :].astype(bf16)            # [1, 512]
    wv = np.ascontiguousarray(np.asarray(W_vocab, f32).T).astype(bf16)  # [512,10000]

    # col 2R is an epsilon column (-55 -> exp ~ 1.3e-24): keeps every row's
    # exp-sum nonzero so 1/ssum stays finite on fully-masked (off-diagonal)
    # rows; abd only consumes cols 0:2R so it never reaches vo.
    maskq = np.full((BL, 4, 2 * R + 1), NEG, f32)
    maskq[:, :, 2 * R] = -55.0
    for j in range(4):
        maskq[2 * j, j, 0:R] = 0.0
        maskq[2 * j + 1, j, R:2 * R] = 0.0
    maskq = maskq.astype(bf16)
    idb = np.eye(128, dtype=f32).astype(bf16)
    ones8 = np.ones((1, BL), f32).astype(bf16)

    # teacher-forced input embeddings: y_seq[t] = emb[ans[:, t-1]], y_seq[0]=emb[1]
    idx = np.concatenate([np.full((B, 1), START_IDX, np.int64), ans[:, :-1]], 1)
    y_emb = emb[idx]                                            # [B,T,EMB]

    def chunked(a, rows=128):
        """[K, N] -> [128, K//128, N] (device layout, partition-major),
        zero-padding K up to a multiple of 128."""
        a = np.asarray(a)
        k, n = a.shape
        nch = -(-k // rows)
        if k < nch * rows:
            pad = np.zeros((nch * rows - k, n), a.dtype)
            a = np.concatenate([a, pad], 0)
        return np.ascontiguousarray(
            a.reshape(nch, rows, n).transpose(1, 0, 2))

    shared = {
        "wyq": chunked(wyq), "wrq": chunked(wrq), "whq": chunked(whq),
        "wcq": chunked(wcq), "wgq": chunked(wgq), "watq": chunked(watq),
        "wuaq": chunked(wuaq), "wuhq": chunked(wuhq), "buq": buq,
        "wv": chunked(wv), "maskq": maskq, "idb": idb, "ones8": ones8,
    }
    in_maps = []
    for c in range(NCORES):
        s = slice(c * BL, (c + 1) * BL)
        ye = y_emb[s].transpose(1, 0, 2).reshape(ROWS, EMB)     # [(t,b), EMB]
        yT = np.concatenate([ye.T, np.ones((1, ROWS), f32)], 0).astype(bf16)
        lT = np.ascontiguousarray(L[s].reshape(BL * R, LOCAL).T).astype(bf16)
        qT = np.ascontiguousarray(q[s].T).astype(bf16)          # [512,8]
        gT = np.concatenate([g[s].T, np.ones((1, BL), f32)], 0).astype(bf16)
        d = {"yT": chunked(yT), "lT": chunked(lT), "qT": chunked(qT),
             "gT": chunked(gT)}
        d.update(shared)
        in_maps.append(d)
    return in_maps


def build_nc():
    """Build the per-core Bass graph (identical on all 8 cores)."""
    from concourse import bacc, tile
    import concourse.mybir as mybir

    f32 = mybir.dt.float32
    bf16 = mybir.dt.bfloat16
    AF = mybir.ActivationFunctionType
    ALU = mybir.AluOpType

    nc = bacc.Bacc("TRN2", target_bir_lowering=False, debug=False,
                   num_devices=NCORES)

    def dparam(name, shape, dt=bf16):
        return nc.dram_tensor(name, list(shape), dt, kind="ExternalInput").ap()

    yT_d = dparam("yT", [128, 3, 256])
    wyq_d = dparam("wyq", [128, 3, 2048])
    wrq_d = dparam("wrq", [128, 8, 2048])
    lT_d = dparam("lT", [128, 8, BL * R])
    watq_d = dparam("watq", [128, 8, 512])
    wuaq_d = dparam("wuaq", [128, 8, 512])
    wuhq_d = dparam("wuhq", [128, 4, 512])
    qT_d = dparam("qT", [128, 4, BL])
    whq_d = dparam("whq", [128, 4, 512])
    wcq_d = dparam("wcq", [128, 4, 512])
    gT_d = dparam("gT", [128, 17, BL])
    wgq_d = dparam("wgq", [128, 17, 512])
    buq_d = dparam("buq", [1, 512])
    wv_d = dparam("wv", [128, 4, VOCAB])
    mask_d = dparam("maskq", [BL, 4, 2 * R + 1])
    idb_d = dparam("idb", [128, 128])
    ones8_d = dparam("ones8", [1, BL])
    out_d = nc.dram_tensor("out", [ROWS, VOCAB], bf16, kind="ExternalOutput").ap()

    mm = nc.tensor.matmul
    ADD, MULT = ALU.add, ALU.mult
    stt = nc.vector.scalar_tensor_tensor
    vcp = nc.vector.tensor_copy
    P104 = 104  # 3*32 + 8: spans all four quad groups

    with tile.TileContext(nc) as tc:
        import contextlib
        stack = contextlib.ExitStack()
        with stack:
            pers = stack.enter_context(tc.tile_pool(name="pers", bufs=1))
            stb = stack.enter_context(tc.tile_pool(name="stb", bufs=2))
            gyp = stack.enter_context(tc.tile_pool(name="gyp", bufs=2))
            gyd = stack.enter_context(tc.tile_pool(name="gyd", bufs=1, space="DRAM"))
            pg = stack.enter_context(tc.tile_pool(name="pg", bufs=2, space="PSUM"))
            ptp = stack.enter_context(tc.tile_pool(name="ptp", bufs=2, space="PSUM"))

            dma = nc.sync.dma_start

            # ---- persistent SBUF ----
            wr_sb = pers.tile([128, 8, 2048], bf16)
            wuh_sb = pers.tile([128, 4, 512], bf16)
            apT_sb = pers.tile([128, 4, BL * R], bf16)
            pbd_sb = pers.tile([128, 4, 512], bf16)
            otb_sb = pers.tile([128, 4, T + 1, BL], bf16)
            wv_sb = pers.tile([128, 4, VOCAB], bf16)
            cq_sb = pers.tile([128, 128], f32)
            abd_sb = pers.tile([128, 4, 32], bf16)
            mask_sb = pers.tile([BL, 4, 2 * R + 1], bf16)
            idb_sb = pers.tile([128, 128], bf16)
            gy_dram = gyd.tile([ROWS, 2048], bf16)

            dma(out=mask_sb[:, :, :], in_=mask_d[:, :, :])
            dma(out=idb_sb[:, :], in_=idb_d[:, :])
            dma(out=abd_sb[2 * R:2 * R + 1, 0, 0:BL], in_=ones8_d[:, :])
            dma(out=pbd_sb[2 * R:2 * R + 1, 0, :], in_=buq_d[:, :])
            dma(out=wuh_sb[:, :, :], in_=wuhq_d[:, :, :])
            dma(out=wr_sb[:, :, :], in_=wrq_d[:, :, :])

            id8 = idb_sb[0:8, 0:8]
            idT = idb_sb[0:P104, 0:128]   # zero-padded transpose identity

            def quad_mm(out_t, lhsT, rhs, j, start, stop):
                mm(out_t[32 * j:32 * j + BL, :], lhsT, rhs, start=start,
                   stop=stop, skip_group_check=True, tile_position=(0, 32 * j))

            def transpose104(in_ap, src_pool=None):
                """[104, F] -> ptp tile [128, 4, 32] (cols 32k+b valid)."""
                tp = ptp.tile([128, 4, 32], bf16, name="tp", tag="tp")
                nP = in_ap.shape[0]
                nc.tensor.transpose(tp[0:in_ap.shape[1], :, :], in_ap,
                                    idb_sb[0:nP, 0:128])
                return tp

            # ---- preamble (all DMA issues hoisted before compute) ----
            hT_sb = None
            with tc.tile_pool(name="pre", bufs=1) as pre, \
                 tc.tile_pool(name="pmm", bufs=2, space="PSUM") as pmm:
                q_sb = pre.tile([128, 4, BL], bf16)
                wh_sb = pre.tile([128, 4, 512], bf16)
                wc_sb = pre.tile([128, 4, 512], bf16)
                y_sb = pre.tile([128, 3, 256], bf16)
                wy_sb = pre.tile([128, 3, 2048], bf16)
                lT_sb = pre.tile([128, 8, BL * R], bf16)
                wat_sb = pre.tile([128, 8, 512], bf16, tag="wab")
                wua_sb = pre.tile([128, 8, 512], bf16, tag="wab")
                g_sb = pre.tile([128, 17, BL], bf16)
                wg_sb = pre.tile([128, 17, 512], bf16)
                dma(out=q_sb[:, :, :], in_=qT_d[:, :, :])
                dma(out=wh_sb[:, :, :], in_=whq_d[:, :, :])
                dma(out=wc_sb[:, :, :], in_=wcq_d[:, :, :])
                dma(out=y_sb[:, :, :], in_=yT_d[:, :, :])
                dma(out=wy_sb[:, :, :], in_=wyq_d[:, :, :])
                dma(out=lT_sb[:, :, :], in_=lT_d[:, :, :])
                dma(out=wat_sb[:, :, :], in_=watq_d[:, :, :])
                dma(out=wua_sb[:, :, :], in_=wuaq_d[:, :, :])
                dma(out=g_sb[:, :, :], in_=gT_d[:, :, :])
                dma(out=wg_sb[:, :, :], in_=wgq_d[:, :, :])

                # h0 / c0 (quad)
                hq0 = pmm.tile([128, 128], f32, name="hq0", tag="mmq")
                cq0 = pmm.tile([128, 128], f32, name="cq0", tag="mmq")
                for dst, w_sb in ((hq0, wh_sb), (cq0, wc_sb)):
                    for k in range(4):
                        for j in range(4):
                            quad_mm(dst, q_sb[:, k, :],
                                    w_sb[:, k, 128 * j:128 * (j + 1)], j,
                                    start=(k == 0), stop=(k == 3))
                h_sb = stb.tile([128, 128], bf16, name="h", tag="h")
                vcp(h_sb[0:P104, :], hq0[0:P104, :])
                vcp(cq_sb[0:P104, :], cq0[0:P104, :])
                tp = transpose104(h_sb[0:P104, :])
                hT_sb = stb.tile([128, 4, 32], bf16, name="hT", tag="hT")
                vcp(hT_sb[:, :, 0:BL], tp[:, :, 0:BL])

                # gates_y -> DRAM scratch (bf16, quad-permuted cols)
                for m in range(2):
                    for nb in range(4):
                        ps = pmm.tile([128, 512], f32, name="gyps", tag="mmq")
                        ns = slice(512 * nb, 512 * (nb + 1))
                        for k in range(3):
                            mm(ps[:, :], y_sb[:, k, 128 * m:128 * (m + 1)],
                               wy_sb[:, k, ns], start=(k == 0), stop=(k == 2))
                        lg = stb.tile([128, 512], bf16, name="lg", tag="lg", bufs=2)
                        vcp(lg[:, :], ps[:, :])
                        dma(out=gy_dram[128 * m:128 * (m + 1), ns], in_=lg[:, :])

                # apT + pbd (shared lT)
                for hk in range(4):
                    ps = pmm.tile([128, BL * R], f32, name="apps", tag="mmq")
                    for k in range(8):
                        mm(ps[:, :], wat_sb[:, k, 128 * hk:128 * (hk + 1)],
                           lT_sb[:, k, :], start=(k == 0), stop=(k == 7))
                    vcp(apT_sb[:, hk, :], ps[:, :])
                for mt in range(4):
                    ps = pmm.tile([128, 512], f32, name="pbps", tag="mmq")
                    for k in range(8):
                        mm(ps[0:2 * R, :],
                           lT_sb[:, k, 2 * R * mt:2 * R * (mt + 1)],
                           wua_sb[:, k, :], start=(k == 0), stop=(k == 7))
                    vcp(pbd_sb[0:2 * R, mt, :], ps[0:2 * R, :])

                # o0 (plain [8,512] then 4 narrow transposes into otb slot 0)
                o0ps = pmm.tile([BL, 512], f32, name="o0ps", tag="mmq")
                for k in range(17):
                    mm(o0ps[:, :], g_sb[:, k, :], wg_sb[:, k, :],
                       start=(k == 0), stop=(k == 16))
                o0_sb = stb.tile([BL, 512], bf16, name="o0", tag="o0")
                vcp(o0_sb[:, :], o0ps[:, :])
                tp0 = ptp.tile([128, 4, 32], bf16, name="tp0", tag="tp")
                for k in range(4):
                    nc.tensor.transpose(tp0[:, k, :],
                                        o0_sb[:, 128 * k:128 * (k + 1)],
                                        idb_sb[0:8, 0:32])
                vcp(otb_sb[:, :, 0, :], tp0[:, :, 0:BL])

                # gates(0): teacher-forced part computed directly from y/wy
                # (t=0 columns of yT) -- no DRAM roundtrip on the start path
                gq = pg.tile([128, 512], f32, name="gq", tag="gq")
                for k in range(3):
                    for j in range(4):
                        quad_mm(gq, y_sb[:, k, 0:BL],
                                wy_sb[:, k, 512 * j:512 * (j + 1)], j,
                                start=(k == 0), stop=False)
                for k in range(4):
                    for j in range(4):
                        quad_mm(gq, hT_sb[:, k, 0:BL],
                                wr_sb[:, 4 + k, 512 * j:512 * (j + 1)], j,
                                start=False, stop=False)
                for k in range(4):
                    for j in range(4):
                        quad_mm(gq, otb_sb[:, k, 0, :],
                                wr_sb[:, k, 512 * j:512 * (j + 1)], j,
                                start=False, stop=(k == 3))

            # vocab weight loads: 2 issued per step over the first 16 steps
            wv_pairs = [(vc, k) for vc in range(8) for k in range(4)]

            pe1 = stack.enter_context(tc.tile_pool(name="pe1", bufs=1, space="PSUM"))
            pv = stack.enter_context(tc.tile_pool(name="pv", bufs=1, space="PSUM"))
            pvoc = stack.enter_context(tc.tile_pool(name="pvoc", bufs=2, space="PSUM"))

            # vocab units: (m_tile, vchunk, col_off, n_size); 3 units share one
            # [128, 1250] staging tile and one out-DMA per (m, vchunk)
            nmm = [(0, 512), (512, 512), (1024, 226)]
            units = [(m, vc, nb, nsz)
                     for m in range(2) for vc in range(8) for nb, nsz in nmm]
            _vstage = {"lg": None}

            def vocab_unit(m, vc, nb, nsz, eng=0):
                ps = pvoc.tile([128, 512], f32, name="vps", tag="vps")
                cb = 1250 * vc + nb
                for k in range(4):
                    mm(ps[:, 0:nsz], otb_sb[:, k, 1 + 16 * m:17 + 16 * m, :],
                       wv_sb[:, k, cb:cb + nsz], start=(k == 0), stop=(k == 3),
                       skip_group_check=True)
                if nb == 0:
                    _vstage["lg"] = stb.tile([128, 1250], bf16, name="lg",
                                             tag="lg", bufs=2)
                lg = _vstage["lg"]
                if eng:
                    vcp(lg[:, nb:nb + nsz], ps[:, 0:nsz])
                else:
                    nc.scalar.copy(lg[:, nb:nb + nsz], ps[:, 0:nsz])
                if nb + nsz == 1250:
                    dma(out=out_d[128 * m:128 * (m + 1),
                                  1250 * vc:1250 * (vc + 1)],
                        in_=lg[:, :])

            # PE keep-warm filler: HAM re-throttles to 1.2 GHz whenever PE
            # duty drops across its 3.4us window; these dead matmuls (into a
            # scratch PSUM, consumed once post-loop) hold K=8/8 through the
            # ACT/DVE-bound cell phase.
            scr_dram = gyd.tile([BL, 512], bf16, name="scr_dram")

            def keep_warm(n):
                ps = pvoc.tile([128, 512], f32, name="vps", tag="vps")
                for k in range(n):
                    mm(ps[0:BL, :], idb_sb[:, 0:BL], wr_sb[:, k, 0:512],
                       start=(k == 0), stop=(k == n - 1),
                       skip_group_check=True)
                return ps

            warm_ps = []

            # ---- recurrence ----
            next_unit = 0
            for t in range(T):
                if t < 16:
                    for vc, k in wv_pairs[2 * t:2 * t + 2]:
                        vs = slice(1250 * vc, 1250 * (vc + 1))
                        dma(out=wv_sb[:, k, vs], in_=wv_d[:, k, vs])
                if t < T - 1:
                    gytq = gyp.tile([BL, 2048], bf16, name="gytq", tag="gytq")
                    dma(out=gytq[:, :],
                        in_=gy_dram[BL * (t + 1):BL * (t + 2), :])

                # fill the ACT/DVE-bound cell window with PE work
                keep_warm(8 if t < 16 else 3)

                # LSTM cell on quad layout; gq cols per group: [g|i|f|og]*128
                tgall = stb.tile([128, 512], bf16, name="tgall", tag="tgall")
                nc.scalar.activation(tgall[0:P104, 0:256], gq[0:P104, 0:256],
                                     AF.Tanh)
                nc.scalar.activation(tgall[0:P104, 256:512],
                                     gq[0:P104, 256:512], AF.Tanh)
                tg = tgall[0:P104, 0:128]
                ti = tgall[0:P104, 128:256]
                tf_ = tgall[0:P104, 256:384]
                tog = tgall[0:P104, 384:512]
                t1 = stb.tile([128, 128], f32, name="t1", tag="t1")
                stt(t1[0:P104, :], ti, 1.0, tg, op0=ADD, op1=MULT)
                m2 = stb.tile([128, 128], f32, name="m2", tag="m2")
                stt(m2[0:P104, :], tf_, 1.0, cq_sb[0:P104, :], op0=ADD, op1=MULT)
                stt(cq_sb[0:P104, :], m2[0:P104, :], 0.5, t1[0:P104, :],
                    op0=MULT, op1=ADD)
                tc2 = stb.tile([128, 128], bf16, name="tc2", tag="tc2")
                nc.scalar.activation(tc2[0:P104, :], cq_sb[0:P104, :], AF.Tanh,
                                     scale=0.5)
                h_sb = stb.tile([128, 128], bf16, name="h", tag="h")
                stt(h_sb[0:P104, :], tog, 1.0, tc2[0:P104, :], op0=ADD, op1=MULT)

                tp = transpose104(h_sb[0:P104, :])
                hT_sb = stb.tile([128, 4, 32], bf16, name="hT", tag="hT")
                vcp(hT_sb[:, :, 0:BL], tp[:, :, 0:BL])

                # attention E first (critical path), masked via id8 wave
                eq = pe1.tile([128, 2 * R + 1], f32, name="eq", tag="eq")
                for j in range(4):
                    quad_mm(eq, id8, mask_sb[0:8, j, :], j, start=True,
                            stop=False)
                for k in range(4):
                    for j in range(4):
                        mm(eq[32 * j:32 * j + BL, 0:2 * R], hT_sb[:, k, 0:BL],
                           apT_sb[:, k, 2 * R * j:2 * R * (j + 1)],
                           start=False, stop=(k == 3), skip_group_check=True,
                           tile_position=(0, 32 * j))

                # PE filler sized to the exp-chain: gates(t+1) gy + h-part
                if t < T - 1:
                    gq_next = pg.tile([128, 512], f32, name="gq", tag="gq")
                    for j in range(4):
                        quad_mm(gq_next, id8, gytq[0:8, 512 * j:512 * (j + 1)],
                                j, start=True, stop=False)
                    for k in range(4):
                        for j in range(4):
                            quad_mm(gq_next, hT_sb[:, k, 0:BL],
                                    wr_sb[:, 4 + k, 512 * j:512 * (j + 1)], j,
                                    start=False, stop=False)
                keep_warm(2)

                expq = stb.tile([128, 2 * R + 1], bf16, name="expq", tag="expq")
                ssum = stb.tile([128, 1], f32, name="ssum", tag="ssum")
                nc.scalar.activation(expq[0:P104, :], eq[0:P104, :], AF.Exp,
                                     accum_out=ssum[0:P104, :])
                rs = stb.tile([128, 1], f32, name="rs", tag="rs")
                nc.vector.reciprocal(rs[0:P104, :], ssum[0:P104, :])
                attq = stb.tile([128, 2 * R], bf16, name="attq", tag="attq")
                nc.vector.tensor_scalar_mul(attq[0:P104, :],
                                            expq[0:P104, 0:2 * R],
                                            rs[0:P104, :])
                tpE = transpose104(attq[0:P104, :])
                vcp(abd_sb[0:2 * R, :, 0:BL], tpE[0:2 * R, :, 0:BL])

                # vo: h-part runs right after attT (hides the abd copy),
                # then b_u + att-weighted P
                vq = pv.tile([128, 128], f32, name="vq", tag="vq")
                for k in range(4):
                    for j in range(4):
                        quad_mm(vq, hT_sb[:, k, 0:BL],
                                wuh_sb[:, k, 128 * j:128 * (j + 1)], j,
                                start=(k == 0), stop=False)
                for m in range(4):
                    kk = 2 * R + 1 if m == 0 else 2 * R
                    for j in range(4):
                        quad_mm(vq, abd_sb[0:kk, m, 0:BL],
                                pbd_sb[0:kk, m, 128 * j:128 * (j + 1)], j,
                                start=False, stop=(m == 3))
                o_sb = stb.tile([128, 128], bf16, name="o", tag="o")
                nc.scalar.activation(o_sb[0:P104, :], vq[0:P104, :], AF.Tanh)

                tpo = transpose104(o_sb[0:P104, :])
                vcp(otb_sb[:, :, t + 1, :], tpo[:, :, 0:BL])

                # gates(t+1): o-part
                if t < T - 1:
                    for k in range(4):
                        for j in range(4):
                            quad_mm(gq_next, otb_sb[:, k, t + 1, :],
                                    wr_sb[:, k, 512 * j:512 * (j + 1)], j,
                                    start=False, stop=(k == 3))
                    gq = gq_next

                # M-tile-0 vocab units fill the next cell's PE-idle window
                if t >= 16:
                    n_units = 2 if t >= 24 else 1
                    for _ in range(n_units):
                        vocab_unit(*units[next_unit], eng=next_unit % 2)
                        next_unit += 1

            # consume the last keep-warm scratch so the chain stays live
            wlast = keep_warm(1)
            wsb = stb.tile([BL, 512], bf16, name="wsb", tag="o0")
            vcp(wsb[:, :], wlast[0:BL, :])
            dma(out=scr_dram[:, :], in_=wsb[:, :])

            # ---- vocab M-tile 1 ----
            for u in range(next_unit, 48):
                vocab_unit(*units[u], eng=u % 2)

    nc.compile()
    return nc


_STATE = {}


def kernel(**inputs):
    from concourse.bass_utils import run_bass_kernel_spmd

    in_maps = prep_inputs(**inputs)
    if "nc" not in _STATE:
        _STATE["nc"] = build_nc()
    nc = _STATE["nc"]
    res = run_bass_kernel_spmd(nc, in_maps, core_ids=list(range(NCORES)))
    bv = np.asarray(inputs["b_vocab"], np.float32)
    full = np.empty((B, T, VOCAB), np.float32)
    for c in range(NCORES):
        o = res.results[c]["out"].astype(np.float32) + bv[None, :]
        full[c * BL:(c + 1) * BL] = o.reshape(T, BL, VOCAB).transpose(1, 0, 2)
    return full
